# revision 64
# baseline (speedup 1.0000x reference)
"""NequIP GNN message-passing kernel for 8 Trainium2 NeuronCores.

Strategy (receiver-sharded graph parallelism per the sharding hint):
- Host: LPT-assigns the 8192 nodes to 64 (core, window) bins of 128 nodes,
  balancing in-edge counts. Each core owns 8 windows = 1024 nodes and the
  edges pointing at them, sorted by window, padded to 128-edge tiles. Edge
  geometry (spherical harmonics * cutoff, Bessel basis) is precomputed on
  host; all heavy per-edge/channel compute runs on device.
- Device (per layer), v3 (bf16 + PE-fold tensor product):
  * gather of sender features in bf16 (640-col padded table, 1280B/edge)
  * radial MLP + w3 projection on TensorE in bf16, software-pipelined one
    edge-group ahead of the tensor product
  * tensor product: P-blocks (xg*w, tensor_tensor @2x bf16, DVE+GpSimd),
    then EVERY Clebsch-Gordan triple becomes a PE matmul: stationary =
    signed-|cg|-scaled P-block pairs (tensor_scalar @4x bf16), moving =
    host-shipped sh_j-scaled selector variants (smatv), accumulated
    directly into the windowed aggregation PSUM. No per-edge messages,
    no DVE FMA chains, no separate segment-sum stage.
  * l2=0 paths fold their cg into w3 columns host-side (j=0 variant)
  * self-interaction + gate + skip on TensorE (bf16 weights)
- Layer 1: per-k path products + one fold matmul per k against cg-baked
  selector variants.
"""
import math
import numpy as np

try:
    from ml_dtypes import bfloat16 as np_bf16
except ImportError:  # pragma: no cover
    import jax.numpy as _jnp
    np_bf16 = _jnp.bfloat16

# ---------------- model constants ----------------
N_NODES, N_EDGES = 8192, 131072
C, H, NRAD = 64, 64, 8
R_MAX, AVG_NN = 5.0, 16.0
NCORES, NPC = 8, 1024
NW = NPC // 128
F = 9 * C
GCOL2 = 640                 # bf16 gather row for layer 2 (576 + 64 pad)
GCOL1 = 128                 # bf16 gather row for layer 1 (64 + 64 pad)
LS = (0, 1, 2)
PATHS = [(l1, l2, l3) for l1 in LS for l2 in LS for l3 in LS
         if abs(l1 - l2) <= l3 <= l1 + l2]
LOFF = {0: 0, 1: 1, 2: 4}
J_OF_L2 = {0: [0], 1: [1, 2, 3], 2: [4, 5, 6, 7, 8]}
BP = 8                      # tile batch for DVE ops



# ---------------- real Clebsch-Gordan coefficients ----------------
def _cg_scalar(j1, m1, j2, m2, j3, m3):
    f = math.factorial
    if m1 + m2 != m3:
        return 0.0
    pre = ((2*j3+1) * f(j1+j2-j3) * f(j1-j2+j3) * f(-j1+j2+j3)
           / f(j1+j2+j3+1)) ** 0.5
    pre *= (f(j1+m1)*f(j1-m1)*f(j2+m2)*f(j2-m2)*f(j3+m3)*f(j3-m3)) ** 0.5
    s = 0.0
    for k in range(max(0, j2-j3-m1, j1+m2-j3), min(j1+j2-j3, j1-m1, j2+m2)+1):
        s += (-1)**k / (f(k)*f(j1+j2-j3-k)*f(j1-m1-k)
                        * f(j2+m2-k)*f(j3-j2+m1+k)*f(j3-j1-m2+k))
    return pre * s


def _U_real(l):
    U = np.zeros((2*l+1, 2*l+1), dtype=complex)
    s2 = 2 ** -0.5
    for m in range(-l, l+1):
        if m > 0:
            U[m+l, m+l] = (-1)**m * s2
            U[m+l, -m+l] = s2
        elif m == 0:
            U[l, l] = 1.0
        else:
            U[m+l, m+l] = 1j*s2
            U[m+l, -m+l] = -1j*(-1)**(-m)*s2
    return U


def _real_cg(l1, l2, l3):
    Cc = np.zeros((2*l1+1, 2*l2+1, 2*l3+1))
    for i1, m1 in enumerate(range(-l1, l1+1)):
        for i2, m2 in enumerate(range(-l2, l2+1)):
            m3 = m1 + m2
            if abs(m3) <= l3:
                Cc[i1, i2, m3+l3] = _cg_scalar(l1, m1, l2, m2, l3, m3)
    U1, U2, U3 = _U_real(l1), _U_real(l2), _U_real(l3)
    W = np.einsum('ia,jb,kc,abc->ijk', U1.conj(), U2.conj(), U3,
                  Cc.astype(complex))
    W = W.real if np.linalg.norm(W.real) >= np.linalg.norm(W.imag) else W.imag
    W = W / np.linalg.norm(W) * (2*l3+1) ** 0.5
    return np.asarray(W, dtype=np.float64)


CGS = [_real_cg(*p) for p in PATHS]


def build_tp_tables(path_ids):
    """Static TP structure, l2-grouped (see v1 docstring)."""
    groups = []
    for l2 in (0, 1, 2):
        ps = [p for p in path_ids if PATHS[p][1] == l2]
        blocks, block_of = [], {}
        for p in ps:
            l1 = PATHS[p][0]
            for i in range(2*l1+1):
                block_of[(p, i)] = len(blocks)
                blocks.append((p, LOFF[l1] + i))
        triples = []
        for p in ps:
            l1, _, l3 = PATHS[p]
            cg = CGS[p]
            for i in range(2*l1+1):
                for j in range(2*l2+1):
                    for k in range(2*l3+1):
                        v = cg[i, j, k]
                        if abs(v) > 1e-12:
                            triples.append((LOFF[l2] + j, block_of[(p, i)],
                                            LOFF[l3] + k, float(v)))
        groups.append((blocks, J_OF_L2[l2], triples))
    return groups


TP_GROUPS_L2 = build_tp_tables(list(range(15)))

# l2=0 paths: CG is v*delta_ik with a single v per path -> fold v into the
# host-side w3 columns and do the whole l2=0 group as PE matmuls against a
# sh0-scaled selector (smat0). Device then skips zj/triples for group 0.
L20_PATHS = [p for p in range(15) if PATHS[p][1] == 0]       # [0, 3, 9]
L20_CG = {p: float(CGS[p][0, 0, 0]) for p in L20_PATHS}
for _p in L20_PATHS:
    _l1 = PATHS[_p][0]
    _d = np.diag(CGS[_p][:, 0, :])
    assert np.allclose(CGS[_p][:, 0, :], np.diag(_d)), _p
    assert np.allclose(_d, _d[0]), _p


def _build_fold_slots():
    """Per j in 1..8: paired matmul slots for the CG fold. Each slot is
    (m, ev, od): matmul into agg pair-region m; ev/od = (z_local, cg) feed
    k=2m / k=2m+1 via the low/high stationary half. Paired slots are ordered
    first so the first bank-A matmul covers all 128 partitions."""
    out = {}
    for j in range(1, 9):
        gi = 1 if j <= 3 else 2
        _, _, trs = TP_GROUPS_L2[gi]
        bym = {}
        for (tj, z, k, v) in trs:
            if tj == j:
                bym.setdefault(k // 2, ([], []))[k % 2].append((z, float(v)))
        slots = []
        for m in sorted(bym):
            ev, od = bym[m]
            for i in range(max(len(ev), len(od))):
                slots.append((m,
                              ev[i] if i < len(ev) else None,
                              od[i] if i < len(od) else None))
        slots.sort(key=lambda s: (s[1] is None or s[2] is None))
        out[j] = (gi, slots)
    return out


FOLD_SLOTS = _build_fold_slots()

# layer-1 per-k scale table: msgs_k = P_{p(k)} * (cg_k * sh_{j(k)})
# paths with l1=0: (0,0,0)->p0, (0,1,1)->p1, (0,2,2)->p2
L1_PATH_OF_K = [0, 1, 1, 1, 2, 2, 2, 2, 2]
L1_CG_OF_K = [float(CGS[0][0, 0, 0])] + [float(CGS[1][0, j, j]) for j in range(3)] \
    + [float(CGS[2][0, j, j]) for j in range(5)]


# ---------------- host-side graph preprocessing ----------------
def edge_geometry(positions, senders, receivers):
    rel = (positions[receivers] - positions[senders]) / R_MAX
    d = np.linalg.norm(rel, axis=-1)
    u = rel / np.maximum(d, 1e-6)[:, None]
    x, y, z = u[:, 0], u[:, 1], u[:, 2]
    sh = np.empty((len(d), 9), np.float32)
    sh[:, 0] = 1.0
    sh[:, 1] = np.sqrt(3.0) * y
    sh[:, 2] = np.sqrt(3.0) * z
    sh[:, 3] = np.sqrt(3.0) * x
    sh[:, 4] = np.sqrt(15.0) * x * y
    sh[:, 5] = np.sqrt(15.0) * y * z
    sh[:, 6] = np.sqrt(5.0) / 2 * (3 * z * z - 1.0)
    sh[:, 7] = np.sqrt(15.0) * x * z
    sh[:, 8] = np.sqrt(15.0) / 2 * (x * x - y * y)
    freqs = np.arange(1, NRAD + 1, dtype=np.float64)
    xr = np.clip(d, 1e-4, 1.0)[:, None].astype(np.float64)
    basis = (np.sqrt(2.0) * np.sin(freqs * np.pi * xr) / xr).astype(np.float32)
    cut = (0.5 * (np.cos(np.pi * np.clip(d, 0.0, 1.0)) + 1.0)).astype(np.float32)
    return (sh * cut[:, None]).astype(np.float32), basis


def partition_graph(receivers):
    import heapq
    deg = np.bincount(receivers, minlength=N_NODES)
    order = np.argsort(-deg, kind="stable")
    nbins = NCORES * NW
    load = np.zeros(nbins, np.int64)
    cnt = np.zeros(nbins, np.int64)
    owner = np.empty(N_NODES, np.int32)
    local = np.empty(N_NODES, np.int32)
    heap = [(0, b) for b in range(nbins)]
    heapq.heapify(heap)
    for n in order:
        while True:
            l, b = heapq.heappop(heap)
            if cnt[b] < 128:
                break
        owner[n] = b // NW
        local[n] = (b % NW) * 128 + cnt[b]
        cnt[b] += 1
        load[b] += deg[n]
        if cnt[b] < 128:
            heapq.heappush(heap, (int(load[b]), b))
    nodes_of = np.empty((NCORES, NPC), np.int64)
    for n in range(N_NODES):
        nodes_of[owner[n], local[n]] = n
    return owner, local, nodes_of, int(load.max())


def build_core_edges(receivers, owner, local, tpw):
    T = NW * tpw
    perm = np.full((NCORES, T * 128), -1, np.int64)
    for k in range(NCORES):
        eids = np.where(owner[receivers] == k)[0]
        lr = local[receivers[eids]]
        o = np.argsort(lr, kind="stable")
        eids, lr = eids[o], lr[o]
        w_of = lr // 128
        for w in range(NW):
            sel = eids[w_of == w]
            assert len(sel) <= tpw * 128, "tiles-per-window overflow"
            base = w * tpw * 128
            perm[k, base:base + len(sel)] = sel
    return perm


# ---------------- bass kernel builder (v2, bf16) ----------------
def build_layer_kernel(layer2, T, debug=False):
    import concourse.bass as bass
    import concourse.bacc as bacc
    import concourse.tile as tile
    import concourse.mybir as mybir
    from contextlib import ExitStack

    fp32 = mybir.dt.float32
    bf16 = mybir.dt.bfloat16
    AF = mybir.ActivationFunctionType
    ALU = mybir.AluOpType

    NPATH = 15 if layer2 else 3
    GCOL = GCOL2 if layer2 else GCOL1
    WCOL = NPATH * C
    E_PAD = T * 128
    NG = T // BP
    assert T % BP == 0 and T % NW == 0
    tpw = T // NW
    MAXBLK = max(len(b) for b, _, _ in TP_GROUPS_L2) if layer2 else 0

    nc = bacc.Bacc("TRN2", target_bir_lowering=False)

    ftab = nc.dram_tensor("ftab", [N_NODES, GCOL], bf16, kind="ExternalInput")
    sidx = nc.dram_tensor("sidx", [128, E_PAD // 16], mybir.dt.int16,
                          kind="ExternalInput")
    basT_d = nc.dram_tensor("basisT", [NG, 8, BP * 128], bf16,
                            kind="ExternalInput")
    # 9 selector variants: sh_j-scaled (L2) / cg*sh_j(k)-scaled per k (L1)
    smatv_d = nc.dram_tensor("smatv", [128, T, 9, 128], bf16,
                             kind="ExternalInput")
    oldT_d = nc.dram_tensor("oldT", [64, 9 * NPC], fp32, kind="ExternalInput")
    w1_d = nc.dram_tensor("w1", [8, H], bf16, kind="ExternalInput")
    b1_d = nc.dram_tensor("b1", [H, 1], fp32, kind="ExternalInput")
    w2_d = nc.dram_tensor("w2", [H, H], bf16, kind="ExternalInput")
    b2_d = nc.dram_tensor("b2", [H, 1], fp32, kind="ExternalInput")
    w3_d = nc.dram_tensor("w3", [H, WCOL], bf16, kind="ExternalInput")
    lin_d = [nc.dram_tensor(f"lin{l}", [C, C], bf16, kind="ExternalInput")
             for l in range(3)]
    gw_d = [nc.dram_tensor(f"gw{l}", [C, C], bf16, kind="ExternalInput")
            for l in range(2)]
    gb_d = [nc.dram_tensor(f"gb{l}", [C, 1], fp32, kind="ExternalInput")
            for l in range(2)]
    newT_d = nc.dram_tensor("newT", [64, 9 * NPC], fp32,
                            kind="ExternalOutput")
    if debug:
        dbg_xg = nc.dram_tensor("dbg_xg", [128, BP, GCOL], fp32,
                                kind="ExternalOutput")
        dbg_w = nc.dram_tensor("dbg_w", [128, BP, WCOL], fp32,
                               kind="ExternalOutput")
        dbg_msgs = nc.dram_tensor("dbg_msgs", [128, BP, F], fp32,
                                  kind="ExternalOutput")
        dbg_agg = nc.dram_tensor("dbg_agg", [128, NW, 5 * 128], fp32,
                                 kind="ExternalOutput")
        dbg_h2 = nc.dram_tensor("dbg_h2", [H, BP * 128], fp32,
                                kind="ExternalOutput")

    with tile.TileContext(nc) as tc, ExitStack() as ctx:
        consts = ctx.enter_context(tc.tile_pool(name="consts", bufs=1))
        idx_sb = consts.tile([128, E_PAD // 16], mybir.dt.int16)
        nc.sync.dma_start(idx_sb[:], sidx[:])
        w1_sb = consts.tile([8, H], bf16)
        nc.sync.dma_start(w1_sb[:], w1_d[:])
        b1_sb = consts.tile([H, 1], fp32)
        nc.sync.dma_start(b1_sb[:], b1_d[:])
        w2_sb = consts.tile([H, H], bf16)
        nc.sync.dma_start(w2_sb[:], w2_d[:])
        b2_sb = consts.tile([H, 1], fp32)
        nc.sync.dma_start(b2_sb[:], b2_d[:])
        w3_sb = consts.tile([H, WCOL], bf16)
        nc.sync.dma_start(w3_sb[:], w3_d[:])
        # lin weights replicated into both partition halves (pair-layout agg)
        lin_sb = [consts.tile([128, C], bf16, name=f"lin{l}", tag=f"lin{l}")
                  for l in range(3)]
        for l in range(3):
            nc.sync.dma_start(lin_sb[l][0:C, :], lin_d[l][:])
            nc.sync.dma_start(lin_sb[l][C:2 * C, :], lin_d[l][:])
        gw_sb = [consts.tile([C, C], bf16, name=f"gw{l}", tag=f"gw{l}")
                 for l in range(2)]
        gb_sb = [consts.tile([C, 1], fp32, name=f"gb{l}", tag=f"gb{l}")
                 for l in range(2)]
        for l in range(2):
            nc.sync.dma_start(gw_sb[l][:], gw_d[l][:])
            nc.sync.dma_start(gb_sb[l][:], gb_d[l][:])
        agg_sb = consts.tile([128, NW, 5 * 128], bf16)

        iop = ctx.enter_context(tc.tile_pool(name="iop", bufs=2))
        wp = ctx.enter_context(tc.tile_pool(name="wp", bufs=2))
        pp = ctx.enter_context(tc.tile_pool(name="pp", bufs=1))
        zp = ctx.enter_context(tc.tile_pool(name="zp", bufs=6))
        msgp = ctx.enter_context(tc.tile_pool(name="msgp", bufs=1))

        def issue_gather(g, xg):
            nc.gpsimd.dma_gather(
                out_ap=xg[:],
                in_ap=ftab[:],
                idxs_ap=idx_sb[:, g * (BP * 8):(g + 1) * (BP * 8)],
                num_idxs=BP * 128,
                num_idxs_reg=BP * 128,
                elem_size=GCOL,
            )

        with ExitStack() as psctx:
            mlp_ps = psctx.enter_context(
                tc.tile_pool(name="mlp_ps", bufs=2, space="PSUM"))
            wps_ps = psctx.enter_context(
                tc.tile_pool(name="wps_ps", bufs=2, space="PSUM"))
            agg_pool = psctx.enter_context(
                tc.tile_pool(name="agg_ps", bufs=2, space="PSUM"))

            agg_open = {}
            xg_tiles = {}
            xg_tiles[0] = iop.tile([128, BP, GCOL], bf16, name="xg", tag="xg")
            issue_gather(0, xg_tiles[0])
            w_tiles = {}

            def emit_w(g):
                # radial MLP + w3 projection for group g (PE, one group ahead)
                bas = iop.tile([8, BP * 128], bf16, name="bas", tag="bas")
                nc.sync.dma_start(bas[:], basT_d[g, :, :])
                h1s = iop.tile([H, BP * 128], bf16, name="h1s", tag="h1s")
                h2s = iop.tile([H, BP * 128], bf16, name="h2s", tag="h2s")
                for c0 in range(0, BP * 128, 512):
                    h1p = mlp_ps.tile([H, 512], fp32, name="h1p", tag="hps")
                    nc.tensor.matmul(h1p[:], w1_sb[:],
                                     bas[:, c0:c0 + 512], start=True, stop=True)
                    sg = iop.tile([H, 512], fp32, name="sg", tag="sg")
                    nc.scalar.activation(sg[:], h1p[:], AF.Sigmoid,
                                         bias=b1_sb[:, 0:1])
                    nc.vector.scalar_tensor_tensor(
                        out=h1s[:, c0:c0 + 512], in0=h1p[:],
                        scalar=b1_sb[:, 0:1], in1=sg[:],
                        op0=ALU.add, op1=ALU.mult)
                    h2p = mlp_ps.tile([H, 512], fp32, name="h2p", tag="hps")
                    nc.tensor.matmul(h2p[:], w2_sb[:],
                                     h1s[:, c0:c0 + 512], start=True, stop=True)
                    sg2 = iop.tile([H, 512], fp32, name="sg2", tag="sg")
                    nc.scalar.activation(sg2[:], h2p[:], AF.Sigmoid,
                                         bias=b2_sb[:, 0:1])
                    nc.vector.scalar_tensor_tensor(
                        out=h2s[:, c0:c0 + 512], in0=h2p[:],
                        scalar=b2_sb[:, 0:1], in1=sg2[:],
                        op0=ALU.add, op1=ALU.mult)
                w_sb = wp.tile([128, BP, WCOL], bf16, name="w_sb", tag="wsb")
                for bt in range(BP):
                    for ci, c0 in enumerate(range(0, WCOL, 480)):
                        c1 = min(c0 + 480, WCOL)
                        w_ps = wps_ps.tile([128, c1 - c0], fp32, name="w_ps",
                                           tag="wps")
                        nc.tensor.matmul(w_ps[:],
                                         h2s[:, bt * 128:(bt + 1) * 128],
                                         w3_sb[:, c0:c1], start=True, stop=True)
                        nc.scalar.copy(w_sb[:, bt, c0:c1], w_ps[:])
                w_tiles[g] = w_sb

            emit_w(0)

            for g in range(NG):
                t0 = g * BP
                xg = xg_tiles.pop(g)
                if g + 1 < NG:
                    xg_tiles[g + 1] = iop.tile([128, BP, GCOL], bf16, name="xg", tag="xg")
                    issue_gather(g + 1, xg_tiles[g + 1])
                    emit_w(g + 1)
                w_sb = w_tiles.pop(g)
                smtv = iop.tile([128, BP, 9, 128], bf16, tag="smtv")
                nc.sync.dma_start(smtv[:], smatv_d[:, t0:t0 + BP, :, :])


                if layer2:
                    # tensor product, PE-fold form: all CG triples become
                    # matmuls of signed-|cg|-scaled P-blocks against the
                    # sh_j-scaled selector variants, accumulated in PSUM.
                    P_tiles = {}
                    for gi, (blocks, jlist, triples) in enumerate(TP_GROUPS_L2):
                        P_sb = pp.tile([128, BP, len(blocks) * C], bf16,
                                       name="psb", tag=f"psb{gi}")
                        for nb, (p, ig) in enumerate(blocks):
                            eng = (nc.gpsimd if (gi == 2 and ig >= 4)
                                   or (gi == 1 and ig >= 7)
                                   else nc.vector)
                            eng.tensor_tensor(
                                out=P_sb[:, :, nb * C:(nb + 1) * C],
                                in0=xg[:, :, ig * C:(ig + 1) * C],
                                in1=w_sb[:, :, p * C:(p + 1) * C],
                                op=ALU.mult)
                        P_tiles[gi] = P_sb

                    def agg_of(w):
                        if w not in agg_open:
                            agg_open[w] = (
                                agg_pool.tile([128, 5 * 128], fp32,
                                              name="aggps", tag="aggps"),
                                {})
                        return agg_open[w]

                    for j in range(1, 9):
                        gi, slots = FOLD_SLOTS[j]
                        P_sb = P_tiles[gi]
                        for (m, ev, od) in slots:
                            pt = zp.tile([128, BP, 128], bf16,
                                         name="pt", tag="pt")
                            if ev is not None:
                                z, v = ev
                                nc.vector.tensor_scalar_mul(
                                    pt[:, :, 0:C],
                                    P_sb[:, :, z * C:(z + 1) * C], v)
                            if od is not None:
                                z, v = od
                                nc.vector.tensor_scalar_mul(
                                    pt[:, :, C:2 * C],
                                    P_sb[:, :, z * C:(z + 1) * C], v)
                            for bt in range(BP):
                                t = t0 + bt
                                w = t // tpw
                                ps, started = agg_of(w)
                                bank = "B" if m == 4 else "A"
                                if ev is not None and od is not None:
                                    halves = (0, 1)
                                    stat = pt[:, bt, :]
                                    out = ps[:, m * 128:(m + 1) * 128]
                                elif ev is not None:
                                    halves = (0,)
                                    stat = pt[:, bt, 0:C]
                                    out = ps[0:C, m * 128:(m + 1) * 128]
                                else:
                                    halves = (1,)
                                    stat = pt[:, bt, C:2 * C]
                                    out = ps[C:2 * C,
                                             m * 128:(m + 1) * 128]
                                st = not any(started.get((bank, h))
                                             for h in halves)
                                for h in halves:
                                    started[(bank, h)] = True
                                nc.tensor.matmul(
                                    out, stat, smtv[:, bt, j, :],
                                    start=st, stop=False,
                                    skip_group_check=True)
                    # l2=0 group direct (cg folded into w3), closes groups
                    for bt in range(BP):
                        t = t0 + bt
                        w, t_in_w = t // tpw, t % tpw
                        ps, started = agg_of(w)
                        last = t_in_w == tpw - 1
                        for pr in range(5):
                            ncols = 128 if pr < 4 else 64
                            bank = "B" if pr == 4 else "A"
                            halves = (0, 1) if ncols == 128 else (0,)
                            st = not any(started.get((bank, h))
                                         for h in halves)
                            for h in halves:
                                started[(bank, h)] = True
                            nc.tensor.matmul(
                                ps[0:ncols, pr * 128:(pr + 1) * 128],
                                P_tiles[0][:, bt,
                                           pr * 128:pr * 128 + ncols],
                                smtv[:, bt, 0, :], start=st,
                                stop=last and pr >= 3,
                                skip_group_check=True)
                    for bt in range(BP):
                        t = t0 + bt
                        w, t_in_w = t // tpw, t % tpw
                        if t_in_w == tpw - 1:
                            ps, _ = agg_open.pop(w)
                            nc.vector.tensor_copy(out=agg_sb[:, w, :],
                                                  in_=ps[:])
                else:
                    # layer 1: per-k path products, then one fold matmul per
                    # k against the cg-baked selector variant (no msgs)
                    P_sb = pp.tile([128, BP, 9 * C], bf16, tag="psb")
                    for kg in range(9):
                        p = L1_PATH_OF_K[kg]
                        nc.vector.tensor_tensor(
                            out=P_sb[:, :, kg * C:(kg + 1) * C],
                            in0=xg[:, :, 0:C],
                            in1=w_sb[:, :, p * C:(p + 1) * C],
                            op=ALU.mult)
                    for bt in range(BP):
                        t = t0 + bt
                        w, t_in_w = t // tpw, t % tpw
                        if w not in agg_open:
                            agg_open[w] = (agg_pool.tile(
                                [128, 5 * 128], fp32, name="aggps",
                                tag="aggps"), {})
                        ps, started = agg_open[w]
                        last = t_in_w == tpw - 1
                        for kg in range(9):
                            m = kg // 2
                            p0 = (kg % 2) * C
                            bank = "B" if m == 4 else "A"
                            st = not started.get((bank, kg % 2))
                            started[(bank, kg % 2)] = True
                            nc.tensor.matmul(
                                ps[p0:p0 + C, m * 128:(m + 1) * 128],
                                P_sb[:, bt, kg * C:(kg + 1) * C],
                                smtv[:, bt, kg, :], start=st,
                                stop=last and kg >= 7,
                                skip_group_check=True)
                        if last:
                            ps, _ = agg_open.pop(w)
                            nc.vector.tensor_copy(out=agg_sb[:, w, :],
                                                  in_=ps[:])

                if debug and g == 0:
                    dxg = msgp.tile([128, BP, GCOL], fp32, tag="dxg")
                    nc.vector.tensor_copy(out=dxg[:], in_=xg[:])
                    nc.sync.dma_start(dbg_xg[:], dxg[:])
                    dw = msgp.tile([128, BP, WCOL], fp32, tag="dw")
                    nc.vector.tensor_copy(out=dw[:], in_=w_sb[:])
                    nc.sync.dma_start(dbg_w[:], dw[:])
                    dh = msgp.tile([H, BP * 128], fp32, tag="dh")
                    nc.vector.tensor_copy(out=dh[:], in_=h2s[:])
                    nc.sync.dma_start(dbg_h2[:], dh[:])

        if debug:
            dbg_agg_f = None

        # ---------------- per-window node update ----------------
        with ExitStack() as upctx:
            up_ps = upctx.enter_context(
                tc.tile_pool(name="up_ps", bufs=4, space="PSUM"))
            upt = upctx.enter_context(tc.tile_pool(name="upt", bufs=2))
            if debug:
                dbg_agg_t = upt.tile([128, NW, 5 * 128], fp32, tag="dagg")
                nc.vector.tensor_copy(out=dbg_agg_t[:], in_=agg_sb[:])
                nc.sync.dma_start(dbg_agg[:], dbg_agg_t[:])
            def aw_of(w, kg):
                p0 = (kg % 2) * 64
                return agg_sb[p0:p0 + 64, w,
                              (kg // 2) * 128:(kg // 2 + 1) * 128]

            for w in range(NW):
                oldw = upt.tile([64, 9, 128], fp32, tag="oldw")
                nc.sync.dma_start(
                    oldw[:], oldT_d[:, :].rearrange("p (q n) -> p q n",
                                                    q=9)[:, :, w * 128:(w + 1) * 128])
                neww = upt.tile([64, 9, 128], fp32, tag="neww")
                y0p = up_ps.tile([C, 128], fp32, tag="yps")
                nc.tensor.matmul(y0p[:], lin_sb[0][0:C, :], aw_of(w, 0),
                                 start=True, stop=True)
                y0g = upt.tile([C, 128], fp32, tag="y0g")
                nc.scalar.activation(y0g[:], y0p[:], AF.Sigmoid)
                y0s = upt.tile([C, 128], fp32, tag="y0s")
                nc.vector.tensor_tensor(out=y0s[:], in0=y0p[:], in1=y0g[:],
                                        op=ALU.mult)
                x0n = upt.tile([C, 128], fp32, tag="x0n")
                nc.vector.tensor_tensor(out=x0n[:], in0=y0s[:],
                                        in1=oldw[:, 0, :], op=ALU.add)
                x0b = upt.tile([C, 128], bf16, tag="x0b")
                nc.vector.tensor_copy(out=x0b[:], in_=x0n[:])
                nc.vector.tensor_copy(out=neww[:, 0, :], in_=x0n[:])
                gts = []
                for l in (1, 2):
                    gp = up_ps.tile([C, 128], fp32, tag="gps")
                    nc.tensor.matmul(gp[:], gw_sb[l - 1][:], x0b[:],
                                     start=True, stop=True)
                    # (gw stays 64-part; moving x0b is base-0)
                    gt = upt.tile([C, 128], fp32, name=f"gt{l}", tag=f"gt{l}")
                    nc.scalar.activation(gt[:], gp[:], AF.Sigmoid,
                                         bias=gb_sb[l - 1][:, 0:1])
                    gts.append(gt)
                for kg in range(1, 9):
                    l = 1 if kg <= 3 else 2
                    p0 = (kg % 2) * C
                    yp = up_ps.tile([C, 128], fp32, tag="yps")
                    nc.tensor.matmul(yp[:], lin_sb[l][p0:p0 + C, :],
                                     aw_of(w, kg), start=True, stop=True)
                    gy = upt.tile([C, 128], fp32, tag="gy")
                    nc.vector.tensor_tensor(out=gy[:], in0=yp[:],
                                            in1=gts[l - 1][:], op=ALU.mult)
                    nc.vector.tensor_tensor(out=neww[:, kg, :],
                                            in0=gy[:], in1=oldw[:, kg, :],
                                            op=ALU.add)
                nc.sync.dma_start(
                    newT_d[:, :].rearrange("p (q n) -> p q n",
                                           q=9)[:, :, w * 128:(w + 1) * 128],
                    neww[:])

    nc.compile()
    return nc


# ---------------- host orchestration ----------------
def _chunked_T(feats_own):
    """[NPC, 576] -> kg-blocked transposed [64, 9*NPC]."""
    out = np.empty((64, 9 * NPC), np.float32)
    for kg in range(9):
        out[:, kg * NPC:(kg + 1) * NPC] = feats_own[:, kg * 64:(kg + 1) * 64].T
    return out


def _unchunk_T(newT):
    """[64, 9*NPC] -> [NPC, 576]."""
    out = np.empty((NPC, 576), np.float32)
    for kg in range(9):
        out[:, kg * 64:(kg + 1) * 64] = newT[:, kg * NPC:(kg + 1) * NPC].T
    return out


_CACHE = {}


def _prep(positions, senders, receivers):
    key = (senders.tobytes(), receivers.tobytes(), positions.tobytes())
    if _CACHE.get("key") == key:
        return _CACHE["val"]
    sh_eff, basis = edge_geometry(positions, senders, receivers)
    owner, local, nodes_of, _ = partition_graph(receivers)
    # tiles per window: max bin edge count, rounded to tiles, even for BP
    deg_bin = np.zeros(NCORES * NW, np.int64)
    np.add.at(deg_bin, owner[receivers] * NW + local[receivers] // 128, 1)
    tpw = (int(deg_bin.max()) + 127) // 128
    while (NW * tpw) % BP:
        tpw += 1
    T = NW * tpw
    perm = build_core_edges(receivers, owner, local, tpw)

    valid = perm >= 0
    eg = np.where(valid, perm, 0)
    snd = np.where(valid, senders[eg], 0).astype(np.int16)      # [NC, T*128]
    shp_e = sh_eff[eg] * valid[..., None]                        # [NC, T*128, 9]
    bas_e = basis[eg] * valid[..., None]                         # [NC, T*128, 8]
    lr = np.where(valid, local[receivers[eg]], 0)

    NG = T // BP
    inv = np.float32(1.0 / np.sqrt(AVG_NN))
    sidx = np.empty((NCORES, 128, T * 128 // 16), np.int16)
    shp_h = np.empty((NCORES, 128, T, 9), np.float32)
    shp1_h = None  # filled after shp_h below
    bas_h = np.empty((NCORES, NG, 8, BP * 128), np_bf16)
    smat_h = np.zeros((NCORES, 128, T, 128), np_bf16)
    smatv_h = np.zeros((NCORES, 128, T, 9, 128), np_bf16)
    smatv1_h = np.zeros((NCORES, 128, T, 9, 128), np_bf16)
    cg1 = np.asarray(L1_CG_OF_K, np.float32)
    for k in range(NCORES):
        s = snd[k].reshape(T * 8, 16)
        sidx[k] = np.tile(s.T, (8, 1))
        shp_h[k] = shp_e[k].reshape(T, 128, 9).transpose(1, 0, 2)
        if shp1_h is None:
            shp1_h = np.empty((NCORES, 128, T, 9), np.float32)
        shp1_h[k] = shp_h[k] * cg1
        bas_h[k] = bas_e[k].reshape(NG, BP * 128, 8).transpose(0, 2, 1).astype(np_bf16)
        v = valid[k]
        e_slots = np.arange(T * 128)
        p_, t_ = e_slots % 128, e_slots // 128
        cols = lr[k] - (t_ // tpw) * 128
        ok = v & (cols >= 0) & (cols < 128)
        sm = np.zeros((128, T, 128), np.float32)
        sm[p_[ok], t_[ok], cols[ok]] = inv
        smat_h[k] = sm.astype(np_bf16)
        smatv_h[k] = (sm[:, :, None, :]
                      * shp_h[k][:, :, :, None]).astype(np_bf16)
        smatv1_h[k] = (sm[:, :, None, :]
                       * shp1_h[k][:, :, :, None]).astype(np_bf16)
    val = dict(T=T, NG=NG, tpw=tpw, nodes_of=nodes_of, sidx=sidx,
               shp_h=shp_h, shp1_h=shp1_h, bas_h=bas_h, smat_h=smat_h,
               smatv_h=smatv_h, smatv1_h=smatv1_h)
    _CACHE["key"], _CACHE["val"] = key, val
    return val


PROFILE = False          # set True by test.py to capture timing
PROF_NS = []             # per-launch exec_time_ns when PROFILE
TRACE_DIRS = []          # per-launch trace dirs when PROFILE


def _run_layer(nc, pre, table_bf16, oldT_by_core, lw, layer2):
    from concourse.bass_utils import run_bass_kernel_spmd
    in_maps = []
    for k in range(NCORES):
        m = dict(ftab=table_bf16,
                 sidx=pre["sidx"][k],
                 basisT=pre["bas_h"][k],
                 oldT=oldT_by_core[k],
                 w1=lw["w1"], b1=lw["b1"], w2=lw["w2"], b2=lw["b2"],
                 w3=lw["w3"], lin0=lw["lin"][0], lin1=lw["lin"][1],
                 lin2=lw["lin"][2], gw0=lw["gw"][0], gw1=lw["gw"][1],
                 gb0=lw["gb"][0], gb1=lw["gb"][1])
        m["smatv"] = (pre["smatv_h"] if layer2 else pre["smatv1_h"])[k]
        in_maps.append(m)
    if PROFILE:
        import time
        t0 = time.time()
        res = run_bass_kernel_spmd(nc, in_maps, list(range(NCORES)))
        PROF_NS.append(int((time.time() - t0) * 1e9))
    else:
        res = run_bass_kernel_spmd(nc, in_maps, list(range(NCORES)))
    return [res.results[k]["newT"] for k in range(NCORES)]


def _layer_weights(inputs, i, npaths):
    f32 = np.float32
    w3 = np.array(inputs["mlp_w3"][i][:, :npaths * C], f32)
    if npaths == 15:
        # l2=0 paths: CG folded into w3 columns (device skips their zj/triples)
        for p, v in L20_CG.items():
            w3[:, p * C:(p + 1) * C] *= np.float32(v)
    return dict(
        w1=np.ascontiguousarray(inputs["mlp_w1"][i]).astype(np_bf16),
        b1=np.ascontiguousarray(inputs["mlp_b1"][i], f32).reshape(H, 1),
        w2=np.ascontiguousarray(inputs["mlp_w2"][i]).astype(np_bf16),
        b2=np.ascontiguousarray(inputs["mlp_b2"][i], f32).reshape(H, 1),
        w3=np.ascontiguousarray(w3).astype(np_bf16),
        lin=[np.ascontiguousarray(inputs["lin_self"][i, l]).astype(np_bf16)
             for l in range(3)],
        gw=[np.ascontiguousarray(inputs["gate_w"][i, l]).astype(np_bf16)
            for l in range(2)],
        gb=[np.ascontiguousarray(inputs["gate_b"][i, l], f32).reshape(C, 1)
            for l in range(2)],
    )


_KERNEL_CACHE = {}


def _get_kernels(T):
    if T not in _KERNEL_CACHE:
        _KERNEL_CACHE[T] = (build_layer_kernel(False, T),
                            build_layer_kernel(True, T))
    return _KERNEL_CACHE[T]


def kernel(**inputs):
    positions = np.asarray(inputs["positions"], np.float32)
    species = np.asarray(inputs["species"]).astype(np.int64)
    senders = np.asarray(inputs["senders"]).astype(np.int64)
    receivers = np.asarray(inputs["receivers"]).astype(np.int64)

    pre = _prep(positions, senders, receivers)
    T = pre["T"]
    nc1, nc2 = _get_kernels(T)
    nodes_of = pre["nodes_of"]

    # initial features: x0 from species embedding (host; tiny)
    x0 = (np.asarray(inputs["embed"], np.float32)[species]
          @ np.asarray(inputs["w_proj"], np.float32))          # [N, 64]
    table1 = np.zeros((N_NODES, GCOL1), np_bf16)
    table1[:, 0:C] = x0.astype(np_bf16)

    # ---- layer 1 ----
    tbl = np.zeros((N_NODES, F), np.float32)
    tbl[:, 0:C] = x0
    oldT = [_chunked_T(tbl[nodes_of[k]]) for k in range(NCORES)]
    lw = _layer_weights(inputs, 0, 3)
    newT = _run_layer(nc1, pre, table1, oldT, lw, False)

    table2f = np.empty((N_NODES, F), np.float32)
    for k in range(NCORES):
        table2f[nodes_of[k]] = _unchunk_T(newT[k])
    table2 = np.zeros((N_NODES, GCOL2), np_bf16)
    table2[:, 0:F] = table2f.astype(np_bf16)

    # ---- layer 2 ----
    lw = _layer_weights(inputs, 1, 15)
    newT2 = _run_layer(nc2, pre, table2, newT, lw, True)

    table3 = np.empty((N_NODES, F), np.float32)
    for k in range(NCORES):
        table3[nodes_of[k]] = _unchunk_T(newT2[k])

    # ---- output: reorder component-major -> reference layout + alpha ----
    t3 = table3.reshape(N_NODES, 9, C)
    out = np.empty((N_NODES, F), np.float32)
    out[:, 0:64] = t3[:, 0]
    out[:, 64:256] = (0.5 * t3[:, 1:4]).transpose(0, 2, 1).reshape(N_NODES, 192)
    out[:, 256:576] = (0.25 * t3[:, 4:9]).transpose(0, 2, 1).reshape(N_NODES, 320)
    return out


# revision 65
# speedup vs baseline: 18567.2862x; 18567.2862x over previous
"""NequIP GNN message-passing kernel for 8 Trainium2 NeuronCores.

Strategy (receiver-sharded graph parallelism per the sharding hint):
- Host: LPT-assigns the 8192 nodes to 64 (core, window) bins of 128 nodes,
  balancing in-edge counts. Each core owns 8 windows = 1024 nodes and the
  edges pointing at them, sorted by window, padded to 128-edge tiles. Edge
  geometry (spherical harmonics * cutoff, Bessel basis) is precomputed on
  host; all heavy per-edge/channel compute runs on device.
- Device (per layer), v3 (bf16 + PE-fold tensor product):
  * gather of sender features in bf16 (640-col padded table, 1280B/edge)
  * radial MLP + w3 projection on TensorE in bf16, software-pipelined one
    edge-group ahead of the tensor product
  * tensor product: P-blocks (xg*w, tensor_tensor @2x bf16, DVE+GpSimd),
    then EVERY Clebsch-Gordan triple becomes a PE matmul: stationary =
    signed-|cg|-scaled P-block pairs (tensor_scalar @4x bf16), moving =
    host-shipped sh_j-scaled selector variants (smatv), accumulated
    directly into the windowed aggregation PSUM. No per-edge messages,
    no DVE FMA chains, no separate segment-sum stage.
  * l2=0 paths fold their cg into w3 columns host-side (j=0 variant)
  * self-interaction + gate + skip on TensorE (bf16 weights)
- Layer 1: per-k path products + one fold matmul per k against cg-baked
  selector variants.
"""
import math
import numpy as np

try:
    from ml_dtypes import bfloat16 as np_bf16
except ImportError:  # pragma: no cover
    import jax.numpy as _jnp
    np_bf16 = _jnp.bfloat16

# ---------------- model constants ----------------
N_NODES, N_EDGES = 8192, 131072
C, H, NRAD = 64, 64, 8
R_MAX, AVG_NN = 5.0, 16.0
NCORES, NPC = 8, 1024
NW = NPC // 128
F = 9 * C
GCOL2 = 640                 # bf16 gather row for layer 2 (576 + 64 pad)
GCOL1 = 128                 # bf16 gather row for layer 1 (64 + 64 pad)
LS = (0, 1, 2)
PATHS = [(l1, l2, l3) for l1 in LS for l2 in LS for l3 in LS
         if abs(l1 - l2) <= l3 <= l1 + l2]
LOFF = {0: 0, 1: 1, 2: 4}
J_OF_L2 = {0: [0], 1: [1, 2, 3], 2: [4, 5, 6, 7, 8]}
BP = 8                      # tile batch for DVE ops



# ---------------- real Clebsch-Gordan coefficients ----------------
def _cg_scalar(j1, m1, j2, m2, j3, m3):
    f = math.factorial
    if m1 + m2 != m3:
        return 0.0
    pre = ((2*j3+1) * f(j1+j2-j3) * f(j1-j2+j3) * f(-j1+j2+j3)
           / f(j1+j2+j3+1)) ** 0.5
    pre *= (f(j1+m1)*f(j1-m1)*f(j2+m2)*f(j2-m2)*f(j3+m3)*f(j3-m3)) ** 0.5
    s = 0.0
    for k in range(max(0, j2-j3-m1, j1+m2-j3), min(j1+j2-j3, j1-m1, j2+m2)+1):
        s += (-1)**k / (f(k)*f(j1+j2-j3-k)*f(j1-m1-k)
                        * f(j2+m2-k)*f(j3-j2+m1+k)*f(j3-j1-m2+k))
    return pre * s


def _U_real(l):
    U = np.zeros((2*l+1, 2*l+1), dtype=complex)
    s2 = 2 ** -0.5
    for m in range(-l, l+1):
        if m > 0:
            U[m+l, m+l] = (-1)**m * s2
            U[m+l, -m+l] = s2
        elif m == 0:
            U[l, l] = 1.0
        else:
            U[m+l, m+l] = 1j*s2
            U[m+l, -m+l] = -1j*(-1)**(-m)*s2
    return U


def _real_cg(l1, l2, l3):
    Cc = np.zeros((2*l1+1, 2*l2+1, 2*l3+1))
    for i1, m1 in enumerate(range(-l1, l1+1)):
        for i2, m2 in enumerate(range(-l2, l2+1)):
            m3 = m1 + m2
            if abs(m3) <= l3:
                Cc[i1, i2, m3+l3] = _cg_scalar(l1, m1, l2, m2, l3, m3)
    U1, U2, U3 = _U_real(l1), _U_real(l2), _U_real(l3)
    W = np.einsum('ia,jb,kc,abc->ijk', U1.conj(), U2.conj(), U3,
                  Cc.astype(complex))
    W = W.real if np.linalg.norm(W.real) >= np.linalg.norm(W.imag) else W.imag
    W = W / np.linalg.norm(W) * (2*l3+1) ** 0.5
    return np.asarray(W, dtype=np.float64)


CGS = [_real_cg(*p) for p in PATHS]


def build_tp_tables(path_ids):
    """Static TP structure, l2-grouped (see v1 docstring)."""
    groups = []
    for l2 in (0, 1, 2):
        ps = [p for p in path_ids if PATHS[p][1] == l2]
        blocks, block_of = [], {}
        for p in ps:
            l1 = PATHS[p][0]
            for i in range(2*l1+1):
                block_of[(p, i)] = len(blocks)
                blocks.append((p, LOFF[l1] + i))
        triples = []
        for p in ps:
            l1, _, l3 = PATHS[p]
            cg = CGS[p]
            for i in range(2*l1+1):
                for j in range(2*l2+1):
                    for k in range(2*l3+1):
                        v = cg[i, j, k]
                        if abs(v) > 1e-12:
                            triples.append((LOFF[l2] + j, block_of[(p, i)],
                                            LOFF[l3] + k, float(v)))
        groups.append((blocks, J_OF_L2[l2], triples))
    return groups


TP_GROUPS_L2 = build_tp_tables(list(range(15)))

# l2=0 paths: CG is v*delta_ik with a single v per path -> fold v into the
# host-side w3 columns and do the whole l2=0 group as PE matmuls against a
# sh0-scaled selector (smat0). Device then skips zj/triples for group 0.
L20_PATHS = [p for p in range(15) if PATHS[p][1] == 0]       # [0, 3, 9]
L20_CG = {p: float(CGS[p][0, 0, 0]) for p in L20_PATHS}
for _p in L20_PATHS:
    _l1 = PATHS[_p][0]
    _d = np.diag(CGS[_p][:, 0, :])
    assert np.allclose(CGS[_p][:, 0, :], np.diag(_d)), _p
    assert np.allclose(_d, _d[0]), _p


def _build_fold_slots():
    """Per j in 1..8: paired matmul slots for the CG fold. Each slot is
    (m, ev, od): matmul into agg pair-region m; ev/od = (z_local, cg) feed
    k=2m / k=2m+1 via the low/high stationary half. Paired slots are ordered
    first so the first bank-A matmul covers all 128 partitions."""
    out = {}
    for j in range(1, 9):
        gi = 1 if j <= 3 else 2
        _, _, trs = TP_GROUPS_L2[gi]
        bym = {}
        for (tj, z, k, v) in trs:
            if tj == j:
                bym.setdefault(k // 2, ([], []))[k % 2].append((z, float(v)))
        slots = []
        for m in sorted(bym):
            ev, od = bym[m]
            for i in range(max(len(ev), len(od))):
                slots.append((m,
                              ev[i] if i < len(ev) else None,
                              od[i] if i < len(od) else None))
        slots.sort(key=lambda s: (s[1] is None or s[2] is None))
        out[j] = (gi, slots)
    return out


FOLD_SLOTS = _build_fold_slots()

# layer-1 per-k scale table: msgs_k = P_{p(k)} * (cg_k * sh_{j(k)})
# paths with l1=0: (0,0,0)->p0, (0,1,1)->p1, (0,2,2)->p2
L1_PATH_OF_K = [0, 1, 1, 1, 2, 2, 2, 2, 2]
L1_CG_OF_K = [float(CGS[0][0, 0, 0])] + [float(CGS[1][0, j, j]) for j in range(3)] \
    + [float(CGS[2][0, j, j]) for j in range(5)]


# ---------------- host-side graph preprocessing ----------------
def edge_geometry(positions, senders, receivers):
    rel = (positions[receivers] - positions[senders]) / R_MAX
    d = np.linalg.norm(rel, axis=-1)
    u = rel / np.maximum(d, 1e-6)[:, None]
    x, y, z = u[:, 0], u[:, 1], u[:, 2]
    sh = np.empty((len(d), 9), np.float32)
    sh[:, 0] = 1.0
    sh[:, 1] = np.sqrt(3.0) * y
    sh[:, 2] = np.sqrt(3.0) * z
    sh[:, 3] = np.sqrt(3.0) * x
    sh[:, 4] = np.sqrt(15.0) * x * y
    sh[:, 5] = np.sqrt(15.0) * y * z
    sh[:, 6] = np.sqrt(5.0) / 2 * (3 * z * z - 1.0)
    sh[:, 7] = np.sqrt(15.0) * x * z
    sh[:, 8] = np.sqrt(15.0) / 2 * (x * x - y * y)
    freqs = np.arange(1, NRAD + 1, dtype=np.float64)
    xr = np.clip(d, 1e-4, 1.0)[:, None].astype(np.float64)
    basis = (np.sqrt(2.0) * np.sin(freqs * np.pi * xr) / xr).astype(np.float32)
    cut = (0.5 * (np.cos(np.pi * np.clip(d, 0.0, 1.0)) + 1.0)).astype(np.float32)
    return (sh * cut[:, None]).astype(np.float32), basis


def partition_graph(receivers):
    import heapq
    deg = np.bincount(receivers, minlength=N_NODES)
    order = np.argsort(-deg, kind="stable")
    nbins = NCORES * NW
    load = np.zeros(nbins, np.int64)
    cnt = np.zeros(nbins, np.int64)
    owner = np.empty(N_NODES, np.int32)
    local = np.empty(N_NODES, np.int32)
    heap = [(0, b) for b in range(nbins)]
    heapq.heapify(heap)
    for n in order:
        while True:
            l, b = heapq.heappop(heap)
            if cnt[b] < 128:
                break
        owner[n] = b // NW
        local[n] = (b % NW) * 128 + cnt[b]
        cnt[b] += 1
        load[b] += deg[n]
        if cnt[b] < 128:
            heapq.heappush(heap, (int(load[b]), b))
    nodes_of = np.empty((NCORES, NPC), np.int64)
    for n in range(N_NODES):
        nodes_of[owner[n], local[n]] = n
    return owner, local, nodes_of, int(load.max())


def build_core_edges(receivers, owner, local, tpw):
    T = NW * tpw
    perm = np.full((NCORES, T * 128), -1, np.int64)
    for k in range(NCORES):
        eids = np.where(owner[receivers] == k)[0]
        lr = local[receivers[eids]]
        o = np.argsort(lr, kind="stable")
        eids, lr = eids[o], lr[o]
        w_of = lr // 128
        for w in range(NW):
            sel = eids[w_of == w]
            assert len(sel) <= tpw * 128, "tiles-per-window overflow"
            base = w * tpw * 128
            perm[k, base:base + len(sel)] = sel
    return perm


# ---------------- bass kernel builder (v2, bf16) ----------------
def build_layer_kernel(layer2, T, debug=False):
    import concourse.bass as bass
    import concourse.bacc as bacc
    import concourse.tile as tile
    import concourse.mybir as mybir
    from contextlib import ExitStack

    fp32 = mybir.dt.float32
    bf16 = mybir.dt.bfloat16
    AF = mybir.ActivationFunctionType
    ALU = mybir.AluOpType

    NPATH = 15 if layer2 else 3
    GCOL = GCOL2 if layer2 else GCOL1
    WCOL = NPATH * C
    E_PAD = T * 128
    NG = T // BP
    assert T % BP == 0 and T % NW == 0
    tpw = T // NW
    MAXBLK = max(len(b) for b, _, _ in TP_GROUPS_L2) if layer2 else 0

    nc = bacc.Bacc("TRN2", target_bir_lowering=False)

    ftab = nc.dram_tensor("ftab", [N_NODES, GCOL], bf16, kind="ExternalInput")
    sidx = nc.dram_tensor("sidx", [128, E_PAD // 16], mybir.dt.int16,
                          kind="ExternalInput")
    basT_d = nc.dram_tensor("basisT", [NG, 8, BP * 128], bf16,
                            kind="ExternalInput")
    # 9 selector variants: sh_j-scaled (L2) / cg*sh_j(k)-scaled per k (L1)
    smatv_d = nc.dram_tensor("smatv", [128, T, 9, 128], bf16,
                             kind="ExternalInput")
    oldT_d = nc.dram_tensor("oldT", [64, 9 * NPC], fp32, kind="ExternalInput")
    w1_d = nc.dram_tensor("w1", [8, H], bf16, kind="ExternalInput")
    b1_d = nc.dram_tensor("b1", [H, 1], fp32, kind="ExternalInput")
    w2_d = nc.dram_tensor("w2", [H, H], bf16, kind="ExternalInput")
    b2_d = nc.dram_tensor("b2", [H, 1], fp32, kind="ExternalInput")
    w3_d = nc.dram_tensor("w3", [H, WCOL], bf16, kind="ExternalInput")
    lin_d = [nc.dram_tensor(f"lin{l}", [C, C], bf16, kind="ExternalInput")
             for l in range(3)]
    gw_d = [nc.dram_tensor(f"gw{l}", [C, C], bf16, kind="ExternalInput")
            for l in range(2)]
    gb_d = [nc.dram_tensor(f"gb{l}", [C, 1], fp32, kind="ExternalInput")
            for l in range(2)]
    newT_d = nc.dram_tensor("newT", [64, 9 * NPC], fp32,
                            kind="ExternalOutput")
    if debug:
        dbg_xg = nc.dram_tensor("dbg_xg", [128, BP, GCOL], fp32,
                                kind="ExternalOutput")
        dbg_w = nc.dram_tensor("dbg_w", [128, BP, WCOL], fp32,
                               kind="ExternalOutput")
        dbg_msgs = nc.dram_tensor("dbg_msgs", [128, BP, F], fp32,
                                  kind="ExternalOutput")
        dbg_agg = nc.dram_tensor("dbg_agg", [128, NW, 5 * 128], fp32,
                                 kind="ExternalOutput")
        dbg_h2 = nc.dram_tensor("dbg_h2", [H, BP * 128], fp32,
                                kind="ExternalOutput")

    with tile.TileContext(nc) as tc, ExitStack() as ctx:
        consts = ctx.enter_context(tc.tile_pool(name="consts", bufs=1))
        idx_sb = consts.tile([128, E_PAD // 16], mybir.dt.int16)
        nc.sync.dma_start(idx_sb[:], sidx[:])
        w1_sb = consts.tile([8, H], bf16)
        nc.sync.dma_start(w1_sb[:], w1_d[:])
        b1_sb = consts.tile([H, 1], fp32)
        nc.sync.dma_start(b1_sb[:], b1_d[:])
        w2_sb = consts.tile([H, H], bf16)
        nc.sync.dma_start(w2_sb[:], w2_d[:])
        b2_sb = consts.tile([H, 1], fp32)
        nc.sync.dma_start(b2_sb[:], b2_d[:])
        w3_sb = consts.tile([H, WCOL], bf16)
        nc.sync.dma_start(w3_sb[:], w3_d[:])
        # lin weights replicated into both partition halves (pair-layout agg)
        lin_sb = [consts.tile([128, C], bf16, name=f"lin{l}", tag=f"lin{l}")
                  for l in range(3)]
        for l in range(3):
            nc.sync.dma_start(lin_sb[l][0:C, :], lin_d[l][:])
            nc.sync.dma_start(lin_sb[l][C:2 * C, :], lin_d[l][:])
        gw_sb = [consts.tile([C, C], bf16, name=f"gw{l}", tag=f"gw{l}")
                 for l in range(2)]
        gb_sb = [consts.tile([C, 1], fp32, name=f"gb{l}", tag=f"gb{l}")
                 for l in range(2)]
        for l in range(2):
            nc.sync.dma_start(gw_sb[l][:], gw_d[l][:])
            nc.sync.dma_start(gb_sb[l][:], gb_d[l][:])
        agg_sb = consts.tile([128, NW, 5 * 128], bf16)

        iop = ctx.enter_context(tc.tile_pool(name="iop", bufs=2))
        wp = ctx.enter_context(tc.tile_pool(name="wp", bufs=2))
        pp = ctx.enter_context(tc.tile_pool(name="pp", bufs=1))
        zp = ctx.enter_context(tc.tile_pool(name="zp", bufs=6))
        msgp = ctx.enter_context(tc.tile_pool(name="msgp", bufs=1))

        def issue_gather(g, xg):
            nc.gpsimd.dma_gather(
                out_ap=xg[:],
                in_ap=ftab[:],
                idxs_ap=idx_sb[:, g * (BP * 8):(g + 1) * (BP * 8)],
                num_idxs=BP * 128,
                num_idxs_reg=BP * 128,
                elem_size=GCOL,
            )

        with ExitStack() as psctx:
            mlp_ps = psctx.enter_context(
                tc.tile_pool(name="mlp_ps", bufs=2, space="PSUM"))
            wps_ps = psctx.enter_context(
                tc.tile_pool(name="wps_ps", bufs=2, space="PSUM"))
            agg_pool = psctx.enter_context(
                tc.tile_pool(name="agg_ps", bufs=2, space="PSUM"))

            upt = ctx.enter_context(tc.tile_pool(name="upt", bufs=2))

            def aw_of(w, kg):
                p0 = (kg % 2) * 64
                return agg_sb[p0:p0 + 64, w,
                              (kg // 2) * 128:(kg // 2 + 1) * 128]

            def emit_update(w, ps):
                # node update for window w, inline at window close; the agg
                # PSUM tile's own regions (partitions 0-63) serve as matmul
                # scratch now that agg_sb holds the aggregation.
                def reg(i):
                    return ps[0:C, i * 128:(i + 1) * 128]
                oldw = upt.tile([64, 9, 128], fp32, name="oldw", tag="oldw")
                nc.sync.dma_start(
                    oldw[:], oldT_d[:, :].rearrange(
                        "p (q n) -> p q n",
                        q=9)[:, :, w * 128:(w + 1) * 128])
                neww = upt.tile([64, 9, 128], fp32, name="neww", tag="neww")
                y0p = reg(0)
                nc.tensor.matmul(y0p, lin_sb[0][0:C, :], aw_of(w, 0),
                                 start=True, stop=True)
                y0g = upt.tile([C, 128], fp32, name="y0g", tag="y0g")
                nc.scalar.activation(y0g[:], y0p, AF.Sigmoid)
                y0s = upt.tile([C, 128], fp32, name="y0s", tag="y0s")
                nc.vector.tensor_tensor(out=y0s[:], in0=y0p, in1=y0g[:],
                                        op=ALU.mult)
                x0n = upt.tile([C, 128], fp32, name="x0n", tag="x0n")
                nc.vector.tensor_tensor(out=x0n[:], in0=y0s[:],
                                        in1=oldw[:, 0, :], op=ALU.add)
                x0b = upt.tile([C, 128], bf16, name="x0b", tag="x0b")
                nc.vector.tensor_copy(out=x0b[:], in_=x0n[:])
                nc.vector.tensor_copy(out=neww[:, 0, :], in_=x0n[:])
                gts = []
                for l in (1, 2):
                    gp = reg(l)
                    nc.tensor.matmul(gp, gw_sb[l - 1][:], x0b[:],
                                     start=True, stop=True)
                    gt = upt.tile([C, 128], fp32, name=f"gt{l}", tag=f"gt{l}")
                    nc.scalar.activation(gt[:], gp, AF.Sigmoid,
                                         bias=gb_sb[l - 1][:, 0:1])
                    gts.append(gt)
                for kg in range(1, 9):
                    l = 1 if kg <= 3 else 2
                    p0 = (kg % 2) * C
                    yp = reg(3 + kg % 2)
                    nc.tensor.matmul(yp, lin_sb[l][p0:p0 + C, :],
                                     aw_of(w, kg), start=True, stop=True)
                    gy = upt.tile([C, 128], fp32, name="gy", tag="gy")
                    nc.vector.tensor_tensor(out=gy[:], in0=yp,
                                            in1=gts[l - 1][:], op=ALU.mult)
                    nc.vector.tensor_tensor(out=neww[:, kg, :],
                                            in0=gy[:], in1=oldw[:, kg, :],
                                            op=ALU.add)
                nc.sync.dma_start(
                    newT_d[:, :].rearrange("p (q n) -> p q n",
                                           q=9)[:, :, w * 128:(w + 1) * 128],
                    neww[:])

            agg_open = {}
            xg_tiles = {}
            xg_tiles[0] = iop.tile([128, BP, GCOL], bf16, name="xg", tag="xg")
            issue_gather(0, xg_tiles[0])
            w_tiles = {}

            def emit_w(g):
                # radial MLP + w3 projection for group g (PE, one group ahead)
                bas = iop.tile([8, BP * 128], bf16, name="bas", tag="bas")
                nc.sync.dma_start(bas[:], basT_d[g, :, :])
                h1s = iop.tile([H, BP * 128], bf16, name="h1s", tag="h1s")
                h2s = iop.tile([H, BP * 128], bf16, name="h2s", tag="h2s")
                for c0 in range(0, BP * 128, 512):
                    h1p = mlp_ps.tile([H, 512], fp32, name="h1p", tag="hps")
                    nc.tensor.matmul(h1p[:], w1_sb[:],
                                     bas[:, c0:c0 + 512], start=True, stop=True)
                    sg = iop.tile([H, 512], fp32, name="sg", tag="sg")
                    nc.scalar.activation(sg[:], h1p[:], AF.Sigmoid,
                                         bias=b1_sb[:, 0:1])
                    nc.vector.scalar_tensor_tensor(
                        out=h1s[:, c0:c0 + 512], in0=h1p[:],
                        scalar=b1_sb[:, 0:1], in1=sg[:],
                        op0=ALU.add, op1=ALU.mult)
                    h2p = mlp_ps.tile([H, 512], fp32, name="h2p", tag="hps")
                    nc.tensor.matmul(h2p[:], w2_sb[:],
                                     h1s[:, c0:c0 + 512], start=True, stop=True)
                    sg2 = iop.tile([H, 512], fp32, name="sg2", tag="sg")
                    nc.scalar.activation(sg2[:], h2p[:], AF.Sigmoid,
                                         bias=b2_sb[:, 0:1])
                    nc.vector.scalar_tensor_tensor(
                        out=h2s[:, c0:c0 + 512], in0=h2p[:],
                        scalar=b2_sb[:, 0:1], in1=sg2[:],
                        op0=ALU.add, op1=ALU.mult)
                w_sb = wp.tile([128, BP, WCOL], bf16, name="w_sb", tag="wsb")
                for bt in range(BP):
                    for ci, c0 in enumerate(range(0, WCOL, 480)):
                        c1 = min(c0 + 480, WCOL)
                        w_ps = wps_ps.tile([128, c1 - c0], fp32, name="w_ps",
                                           tag="wps")
                        nc.tensor.matmul(w_ps[:],
                                         h2s[:, bt * 128:(bt + 1) * 128],
                                         w3_sb[:, c0:c1], start=True, stop=True)
                        nc.scalar.copy(w_sb[:, bt, c0:c1], w_ps[:])
                w_tiles[g] = w_sb

            emit_w(0)

            for g in range(NG):
                t0 = g * BP
                xg = xg_tiles.pop(g)
                if g + 1 < NG:
                    xg_tiles[g + 1] = iop.tile([128, BP, GCOL], bf16, name="xg", tag="xg")
                    issue_gather(g + 1, xg_tiles[g + 1])
                    emit_w(g + 1)
                w_sb = w_tiles.pop(g)
                smtv = iop.tile([128, BP, 9, 128], bf16, tag="smtv")
                nc.sync.dma_start(smtv[:], smatv_d[:, t0:t0 + BP, :, :])


                if layer2:
                    # tensor product, PE-fold form: all CG triples become
                    # matmuls of signed-|cg|-scaled P-blocks against the
                    # sh_j-scaled selector variants, accumulated in PSUM.
                    P_tiles = {}
                    for gi, (blocks, jlist, triples) in enumerate(TP_GROUPS_L2):
                        P_sb = pp.tile([128, BP, len(blocks) * C], bf16,
                                       name="psb", tag=f"psb{gi}")
                        for nb, (p, ig) in enumerate(blocks):
                            eng = (nc.gpsimd if (gi == 2 and ig >= 4)
                                   or (gi == 1 and ig >= 7)
                                   else nc.vector)
                            eng.tensor_tensor(
                                out=P_sb[:, :, nb * C:(nb + 1) * C],
                                in0=xg[:, :, ig * C:(ig + 1) * C],
                                in1=w_sb[:, :, p * C:(p + 1) * C],
                                op=ALU.mult)
                        P_tiles[gi] = P_sb

                    def agg_of(w):
                        if w not in agg_open:
                            agg_open[w] = (
                                agg_pool.tile([128, 5 * 128], fp32,
                                              name="aggps", tag="aggps"),
                                {})
                        return agg_open[w]

                    for j in range(1, 9):
                        gi, slots = FOLD_SLOTS[j]
                        P_sb = P_tiles[gi]
                        for (m, ev, od) in slots:
                            pt = zp.tile([128, BP, 128], bf16,
                                         name="pt", tag="pt")
                            if ev is not None:
                                z, v = ev
                                nc.vector.tensor_scalar_mul(
                                    pt[:, :, 0:C],
                                    P_sb[:, :, z * C:(z + 1) * C], v)
                            if od is not None:
                                z, v = od
                                nc.vector.tensor_scalar_mul(
                                    pt[:, :, C:2 * C],
                                    P_sb[:, :, z * C:(z + 1) * C], v)
                            for bt in range(BP):
                                t = t0 + bt
                                w = t // tpw
                                ps, started = agg_of(w)
                                bank = "B" if m == 4 else "A"
                                if ev is not None and od is not None:
                                    halves = (0, 1)
                                    stat = pt[:, bt, :]
                                    out = ps[:, m * 128:(m + 1) * 128]
                                elif ev is not None:
                                    halves = (0,)
                                    stat = pt[:, bt, 0:C]
                                    out = ps[0:C, m * 128:(m + 1) * 128]
                                else:
                                    halves = (1,)
                                    stat = pt[:, bt, C:2 * C]
                                    out = ps[C:2 * C,
                                             m * 128:(m + 1) * 128]
                                st = not any(started.get((bank, h))
                                             for h in halves)
                                for h in halves:
                                    started[(bank, h)] = True
                                nc.tensor.matmul(
                                    out, stat, smtv[:, bt, j, :],
                                    start=st, stop=False,
                                    skip_group_check=True)
                    # l2=0 group direct (cg folded into w3), closes groups
                    for bt in range(BP):
                        t = t0 + bt
                        w, t_in_w = t // tpw, t % tpw
                        ps, started = agg_of(w)
                        last = t_in_w == tpw - 1
                        for pr in range(5):
                            ncols = 128 if pr < 4 else 64
                            bank = "B" if pr == 4 else "A"
                            halves = (0, 1) if ncols == 128 else (0,)
                            st = not any(started.get((bank, h))
                                         for h in halves)
                            for h in halves:
                                started[(bank, h)] = True
                            nc.tensor.matmul(
                                ps[0:ncols, pr * 128:(pr + 1) * 128],
                                P_tiles[0][:, bt,
                                           pr * 128:pr * 128 + ncols],
                                smtv[:, bt, 0, :], start=st,
                                stop=last and pr >= 3,
                                skip_group_check=True)
                    for bt in range(BP):
                        t = t0 + bt
                        w, t_in_w = t // tpw, t % tpw
                        if t_in_w == tpw - 1:
                            ps, _ = agg_open.pop(w)
                            nc.vector.tensor_copy(out=agg_sb[:, w, :],
                                                  in_=ps[:])
                            emit_update(w, ps)
                else:
                    # layer 1: per-k path products, then one fold matmul per
                    # k against the cg-baked selector variant (no msgs)
                    P_sb = pp.tile([128, BP, 9 * C], bf16, tag="psb")
                    for kg in range(9):
                        p = L1_PATH_OF_K[kg]
                        nc.vector.tensor_tensor(
                            out=P_sb[:, :, kg * C:(kg + 1) * C],
                            in0=xg[:, :, 0:C],
                            in1=w_sb[:, :, p * C:(p + 1) * C],
                            op=ALU.mult)
                    for bt in range(BP):
                        t = t0 + bt
                        w, t_in_w = t // tpw, t % tpw
                        if w not in agg_open:
                            agg_open[w] = (agg_pool.tile(
                                [128, 5 * 128], fp32, name="aggps",
                                tag="aggps"), {})
                        ps, started = agg_open[w]
                        last = t_in_w == tpw - 1
                        for kg in range(9):
                            m = kg // 2
                            p0 = (kg % 2) * C
                            bank = "B" if m == 4 else "A"
                            st = not started.get((bank, kg % 2))
                            started[(bank, kg % 2)] = True
                            nc.tensor.matmul(
                                ps[p0:p0 + C, m * 128:(m + 1) * 128],
                                P_sb[:, bt, kg * C:(kg + 1) * C],
                                smtv[:, bt, kg, :], start=st,
                                stop=last and kg >= 7,
                                skip_group_check=True)
                        if last:
                            ps, _ = agg_open.pop(w)
                            nc.vector.tensor_copy(out=agg_sb[:, w, :],
                                                  in_=ps[:])
                            emit_update(w, ps)

                if debug and g == 0:
                    dxg = msgp.tile([128, BP, GCOL], fp32, tag="dxg")
                    nc.vector.tensor_copy(out=dxg[:], in_=xg[:])
                    nc.sync.dma_start(dbg_xg[:], dxg[:])
                    dw = msgp.tile([128, BP, WCOL], fp32, tag="dw")
                    nc.vector.tensor_copy(out=dw[:], in_=w_sb[:])
                    nc.sync.dma_start(dbg_w[:], dw[:])
                    dh = msgp.tile([H, BP * 128], fp32, tag="dh")
                    nc.vector.tensor_copy(out=dh[:], in_=h2s[:])
                    nc.sync.dma_start(dbg_h2[:], dh[:])

        if debug:
            dbg_agg_f = None

    nc.compile()
    return nc


# ---------------- host orchestration ----------------
def _chunked_T(feats_own):
    """[NPC, 576] -> kg-blocked transposed [64, 9*NPC]."""
    out = np.empty((64, 9 * NPC), np.float32)
    for kg in range(9):
        out[:, kg * NPC:(kg + 1) * NPC] = feats_own[:, kg * 64:(kg + 1) * 64].T
    return out


def _unchunk_T(newT):
    """[64, 9*NPC] -> [NPC, 576]."""
    out = np.empty((NPC, 576), np.float32)
    for kg in range(9):
        out[:, kg * 64:(kg + 1) * 64] = newT[:, kg * NPC:(kg + 1) * NPC].T
    return out


_CACHE = {}


def _prep(positions, senders, receivers):
    key = (senders.tobytes(), receivers.tobytes(), positions.tobytes())
    if _CACHE.get("key") == key:
        return _CACHE["val"]
    sh_eff, basis = edge_geometry(positions, senders, receivers)
    owner, local, nodes_of, _ = partition_graph(receivers)
    # tiles per window: max bin edge count, rounded to tiles, even for BP
    deg_bin = np.zeros(NCORES * NW, np.int64)
    np.add.at(deg_bin, owner[receivers] * NW + local[receivers] // 128, 1)
    tpw = (int(deg_bin.max()) + 127) // 128
    while (NW * tpw) % BP:
        tpw += 1
    T = NW * tpw
    perm = build_core_edges(receivers, owner, local, tpw)

    valid = perm >= 0
    eg = np.where(valid, perm, 0)
    snd = np.where(valid, senders[eg], 0).astype(np.int16)      # [NC, T*128]
    shp_e = sh_eff[eg] * valid[..., None]                        # [NC, T*128, 9]
    bas_e = basis[eg] * valid[..., None]                         # [NC, T*128, 8]
    lr = np.where(valid, local[receivers[eg]], 0)

    NG = T // BP
    inv = np.float32(1.0 / np.sqrt(AVG_NN))
    sidx = np.empty((NCORES, 128, T * 128 // 16), np.int16)
    shp_h = np.empty((NCORES, 128, T, 9), np.float32)
    shp1_h = None  # filled after shp_h below
    bas_h = np.empty((NCORES, NG, 8, BP * 128), np_bf16)
    smat_h = np.zeros((NCORES, 128, T, 128), np_bf16)
    smatv_h = np.zeros((NCORES, 128, T, 9, 128), np_bf16)
    smatv1_h = np.zeros((NCORES, 128, T, 9, 128), np_bf16)
    cg1 = np.asarray(L1_CG_OF_K, np.float32)
    for k in range(NCORES):
        s = snd[k].reshape(T * 8, 16)
        sidx[k] = np.tile(s.T, (8, 1))
        shp_h[k] = shp_e[k].reshape(T, 128, 9).transpose(1, 0, 2)
        if shp1_h is None:
            shp1_h = np.empty((NCORES, 128, T, 9), np.float32)
        shp1_h[k] = shp_h[k] * cg1
        bas_h[k] = bas_e[k].reshape(NG, BP * 128, 8).transpose(0, 2, 1).astype(np_bf16)
        v = valid[k]
        e_slots = np.arange(T * 128)
        p_, t_ = e_slots % 128, e_slots // 128
        cols = lr[k] - (t_ // tpw) * 128
        ok = v & (cols >= 0) & (cols < 128)
        sm = np.zeros((128, T, 128), np.float32)
        sm[p_[ok], t_[ok], cols[ok]] = inv
        smat_h[k] = sm.astype(np_bf16)
        smatv_h[k] = (sm[:, :, None, :]
                      * shp_h[k][:, :, :, None]).astype(np_bf16)
        smatv1_h[k] = (sm[:, :, None, :]
                       * shp1_h[k][:, :, :, None]).astype(np_bf16)
    val = dict(T=T, NG=NG, tpw=tpw, nodes_of=nodes_of, sidx=sidx,
               shp_h=shp_h, shp1_h=shp1_h, bas_h=bas_h, smat_h=smat_h,
               smatv_h=smatv_h, smatv1_h=smatv1_h)
    _CACHE["key"], _CACHE["val"] = key, val
    return val


PROFILE = False          # set True by test.py to capture timing
PROF_NS = []             # per-launch exec_time_ns when PROFILE
TRACE_DIRS = []          # per-launch trace dirs when PROFILE


def _run_layer(nc, pre, table_bf16, oldT_by_core, lw, layer2):
    from concourse.bass_utils import run_bass_kernel_spmd
    in_maps = []
    for k in range(NCORES):
        m = dict(ftab=table_bf16,
                 sidx=pre["sidx"][k],
                 basisT=pre["bas_h"][k],
                 oldT=oldT_by_core[k],
                 w1=lw["w1"], b1=lw["b1"], w2=lw["w2"], b2=lw["b2"],
                 w3=lw["w3"], lin0=lw["lin"][0], lin1=lw["lin"][1],
                 lin2=lw["lin"][2], gw0=lw["gw"][0], gw1=lw["gw"][1],
                 gb0=lw["gb"][0], gb1=lw["gb"][1])
        m["smatv"] = (pre["smatv_h"] if layer2 else pre["smatv1_h"])[k]
        in_maps.append(m)
    if PROFILE:
        import time
        t0 = time.time()
        res = run_bass_kernel_spmd(nc, in_maps, list(range(NCORES)))
        PROF_NS.append(int((time.time() - t0) * 1e9))
    else:
        res = run_bass_kernel_spmd(nc, in_maps, list(range(NCORES)))
    return [res.results[k]["newT"] for k in range(NCORES)]


def _layer_weights(inputs, i, npaths):
    f32 = np.float32
    w3 = np.array(inputs["mlp_w3"][i][:, :npaths * C], f32)
    if npaths == 15:
        # l2=0 paths: CG folded into w3 columns (device skips their zj/triples)
        for p, v in L20_CG.items():
            w3[:, p * C:(p + 1) * C] *= np.float32(v)
    return dict(
        w1=np.ascontiguousarray(inputs["mlp_w1"][i]).astype(np_bf16),
        b1=np.ascontiguousarray(inputs["mlp_b1"][i], f32).reshape(H, 1),
        w2=np.ascontiguousarray(inputs["mlp_w2"][i]).astype(np_bf16),
        b2=np.ascontiguousarray(inputs["mlp_b2"][i], f32).reshape(H, 1),
        w3=np.ascontiguousarray(w3).astype(np_bf16),
        lin=[np.ascontiguousarray(inputs["lin_self"][i, l]).astype(np_bf16)
             for l in range(3)],
        gw=[np.ascontiguousarray(inputs["gate_w"][i, l]).astype(np_bf16)
            for l in range(2)],
        gb=[np.ascontiguousarray(inputs["gate_b"][i, l], f32).reshape(C, 1)
            for l in range(2)],
    )


_KERNEL_CACHE = {}


def _get_kernels(T):
    if T not in _KERNEL_CACHE:
        _KERNEL_CACHE[T] = (build_layer_kernel(False, T),
                            build_layer_kernel(True, T))
    return _KERNEL_CACHE[T]


def kernel(**inputs):
    positions = np.asarray(inputs["positions"], np.float32)
    species = np.asarray(inputs["species"]).astype(np.int64)
    senders = np.asarray(inputs["senders"]).astype(np.int64)
    receivers = np.asarray(inputs["receivers"]).astype(np.int64)

    pre = _prep(positions, senders, receivers)
    T = pre["T"]
    nc1, nc2 = _get_kernels(T)
    nodes_of = pre["nodes_of"]

    # initial features: x0 from species embedding (host; tiny)
    x0 = (np.asarray(inputs["embed"], np.float32)[species]
          @ np.asarray(inputs["w_proj"], np.float32))          # [N, 64]
    table1 = np.zeros((N_NODES, GCOL1), np_bf16)
    table1[:, 0:C] = x0.astype(np_bf16)

    # ---- layer 1 ----
    tbl = np.zeros((N_NODES, F), np.float32)
    tbl[:, 0:C] = x0
    oldT = [_chunked_T(tbl[nodes_of[k]]) for k in range(NCORES)]
    lw = _layer_weights(inputs, 0, 3)
    newT = _run_layer(nc1, pre, table1, oldT, lw, False)

    table2f = np.empty((N_NODES, F), np.float32)
    for k in range(NCORES):
        table2f[nodes_of[k]] = _unchunk_T(newT[k])
    table2 = np.zeros((N_NODES, GCOL2), np_bf16)
    table2[:, 0:F] = table2f.astype(np_bf16)

    # ---- layer 2 ----
    lw = _layer_weights(inputs, 1, 15)
    newT2 = _run_layer(nc2, pre, table2, newT, lw, True)

    table3 = np.empty((N_NODES, F), np.float32)
    for k in range(NCORES):
        table3[nodes_of[k]] = _unchunk_T(newT2[k])

    # ---- output: reorder component-major -> reference layout + alpha ----
    t3 = table3.reshape(N_NODES, 9, C)
    out = np.empty((N_NODES, F), np.float32)
    out[:, 0:64] = t3[:, 0]
    out[:, 64:256] = (0.5 * t3[:, 1:4]).transpose(0, 2, 1).reshape(N_NODES, 192)
    out[:, 256:576] = (0.25 * t3[:, 4:9]).transpose(0, 2, 1).reshape(N_NODES, 320)
    return out


# revision 66
# speedup vs baseline: 19645.2636x; 1.0581x over previous
"""NequIP GNN message-passing kernel for 8 Trainium2 NeuronCores.

Strategy (receiver-sharded graph parallelism per the sharding hint):
- Host: LPT-assigns the 8192 nodes to 64 (core, window) bins of 128 nodes,
  balancing in-edge counts. Each core owns 8 windows = 1024 nodes and the
  edges pointing at them, sorted by window, padded to 128-edge tiles. Edge
  geometry (spherical harmonics * cutoff, Bessel basis) is precomputed on
  host; all heavy per-edge/channel compute runs on device.
- Device (per layer), v3 (bf16 + PE-fold tensor product):
  * gather of sender features in bf16 (640-col padded table, 1280B/edge)
  * radial MLP + w3 projection on TensorE in bf16, software-pipelined one
    edge-group ahead of the tensor product
  * tensor product: P-blocks (xg*w, tensor_tensor @2x bf16, DVE+GpSimd),
    then EVERY Clebsch-Gordan triple becomes a PE matmul: stationary =
    signed-|cg|-scaled P-block pairs (tensor_scalar @4x bf16), moving =
    host-shipped sh_j-scaled selector variants (smatv), accumulated
    directly into the windowed aggregation PSUM. No per-edge messages,
    no DVE FMA chains, no separate segment-sum stage.
  * l2=0 paths fold their cg into w3 columns host-side (j=0 variant)
  * self-interaction + gate + skip on TensorE (bf16 weights)
- Layer 1: per-k path products + one fold matmul per k against cg-baked
  selector variants.
"""
import math
import numpy as np

try:
    from ml_dtypes import bfloat16 as np_bf16
except ImportError:  # pragma: no cover
    import jax.numpy as _jnp
    np_bf16 = _jnp.bfloat16

# ---------------- model constants ----------------
N_NODES, N_EDGES = 8192, 131072
C, H, NRAD = 64, 64, 8
R_MAX, AVG_NN = 5.0, 16.0
NCORES, NPC = 8, 1024
NW = NPC // 128
F = 9 * C
GCOL2 = 640                 # bf16 gather row for layer 2 (576 + 64 pad)
GCOL1 = 128                 # bf16 gather row for layer 1 (64 + 64 pad)
LS = (0, 1, 2)
PATHS = [(l1, l2, l3) for l1 in LS for l2 in LS for l3 in LS
         if abs(l1 - l2) <= l3 <= l1 + l2]
LOFF = {0: 0, 1: 1, 2: 4}
J_OF_L2 = {0: [0], 1: [1, 2, 3], 2: [4, 5, 6, 7, 8]}
BP = 8                      # tile batch for DVE ops



# ---------------- real Clebsch-Gordan coefficients ----------------
def _cg_scalar(j1, m1, j2, m2, j3, m3):
    f = math.factorial
    if m1 + m2 != m3:
        return 0.0
    pre = ((2*j3+1) * f(j1+j2-j3) * f(j1-j2+j3) * f(-j1+j2+j3)
           / f(j1+j2+j3+1)) ** 0.5
    pre *= (f(j1+m1)*f(j1-m1)*f(j2+m2)*f(j2-m2)*f(j3+m3)*f(j3-m3)) ** 0.5
    s = 0.0
    for k in range(max(0, j2-j3-m1, j1+m2-j3), min(j1+j2-j3, j1-m1, j2+m2)+1):
        s += (-1)**k / (f(k)*f(j1+j2-j3-k)*f(j1-m1-k)
                        * f(j2+m2-k)*f(j3-j2+m1+k)*f(j3-j1-m2+k))
    return pre * s


def _U_real(l):
    U = np.zeros((2*l+1, 2*l+1), dtype=complex)
    s2 = 2 ** -0.5
    for m in range(-l, l+1):
        if m > 0:
            U[m+l, m+l] = (-1)**m * s2
            U[m+l, -m+l] = s2
        elif m == 0:
            U[l, l] = 1.0
        else:
            U[m+l, m+l] = 1j*s2
            U[m+l, -m+l] = -1j*(-1)**(-m)*s2
    return U


def _real_cg(l1, l2, l3):
    Cc = np.zeros((2*l1+1, 2*l2+1, 2*l3+1))
    for i1, m1 in enumerate(range(-l1, l1+1)):
        for i2, m2 in enumerate(range(-l2, l2+1)):
            m3 = m1 + m2
            if abs(m3) <= l3:
                Cc[i1, i2, m3+l3] = _cg_scalar(l1, m1, l2, m2, l3, m3)
    U1, U2, U3 = _U_real(l1), _U_real(l2), _U_real(l3)
    W = np.einsum('ia,jb,kc,abc->ijk', U1.conj(), U2.conj(), U3,
                  Cc.astype(complex))
    W = W.real if np.linalg.norm(W.real) >= np.linalg.norm(W.imag) else W.imag
    W = W / np.linalg.norm(W) * (2*l3+1) ** 0.5
    return np.asarray(W, dtype=np.float64)


CGS = [_real_cg(*p) for p in PATHS]


def build_tp_tables(path_ids):
    """Static TP structure, l2-grouped (see v1 docstring)."""
    groups = []
    for l2 in (0, 1, 2):
        ps = [p for p in path_ids if PATHS[p][1] == l2]
        blocks, block_of = [], {}
        for p in ps:
            l1 = PATHS[p][0]
            for i in range(2*l1+1):
                block_of[(p, i)] = len(blocks)
                blocks.append((p, LOFF[l1] + i))
        triples = []
        for p in ps:
            l1, _, l3 = PATHS[p]
            cg = CGS[p]
            for i in range(2*l1+1):
                for j in range(2*l2+1):
                    for k in range(2*l3+1):
                        v = cg[i, j, k]
                        if abs(v) > 1e-12:
                            triples.append((LOFF[l2] + j, block_of[(p, i)],
                                            LOFF[l3] + k, float(v)))
        groups.append((blocks, J_OF_L2[l2], triples))
    return groups


TP_GROUPS_L2 = build_tp_tables(list(range(15)))

# l2=0 paths: CG is v*delta_ik with a single v per path -> fold v into the
# host-side w3 columns and do the whole l2=0 group as PE matmuls against a
# sh0-scaled selector (smat0). Device then skips zj/triples for group 0.
L20_PATHS = [p for p in range(15) if PATHS[p][1] == 0]       # [0, 3, 9]
L20_CG = {p: float(CGS[p][0, 0, 0]) for p in L20_PATHS}
for _p in L20_PATHS:
    _l1 = PATHS[_p][0]
    _d = np.diag(CGS[_p][:, 0, :])
    assert np.allclose(CGS[_p][:, 0, :], np.diag(_d)), _p
    assert np.allclose(_d, _d[0]), _p


def _build_fold_slots():
    """Per j in 1..8: paired matmul slots for the CG fold. Each slot is
    (m, ev, od): matmul into agg pair-region m; ev/od = (z_local, cg) feed
    k=2m / k=2m+1 via the low/high stationary half. Paired slots are ordered
    first so the first bank-A matmul covers all 128 partitions."""
    out = {}
    for j in range(1, 9):
        gi = 1 if j <= 3 else 2
        _, _, trs = TP_GROUPS_L2[gi]
        bym = {}
        for (tj, z, k, v) in trs:
            if tj == j:
                bym.setdefault(k // 2, ([], []))[k % 2].append((z, float(v)))
        slots = []
        for m in sorted(bym):
            ev, od = bym[m]
            for i in range(max(len(ev), len(od))):
                slots.append((m,
                              ev[i] if i < len(ev) else None,
                              od[i] if i < len(od) else None))
        slots.sort(key=lambda s: (s[1] is None or s[2] is None))
        out[j] = (gi, slots)
    return out


FOLD_SLOTS = _build_fold_slots()

# layer-1 per-k scale table: msgs_k = P_{p(k)} * (cg_k * sh_{j(k)})
# paths with l1=0: (0,0,0)->p0, (0,1,1)->p1, (0,2,2)->p2
L1_PATH_OF_K = [0, 1, 1, 1, 2, 2, 2, 2, 2]
L1_CG_OF_K = [float(CGS[0][0, 0, 0])] + [float(CGS[1][0, j, j]) for j in range(3)] \
    + [float(CGS[2][0, j, j]) for j in range(5)]


# ---------------- host-side graph preprocessing ----------------
def edge_geometry(positions, senders, receivers):
    rel = (positions[receivers] - positions[senders]) / R_MAX
    d = np.linalg.norm(rel, axis=-1)
    u = rel / np.maximum(d, 1e-6)[:, None]
    x, y, z = u[:, 0], u[:, 1], u[:, 2]
    sh = np.empty((len(d), 9), np.float32)
    sh[:, 0] = 1.0
    sh[:, 1] = np.sqrt(3.0) * y
    sh[:, 2] = np.sqrt(3.0) * z
    sh[:, 3] = np.sqrt(3.0) * x
    sh[:, 4] = np.sqrt(15.0) * x * y
    sh[:, 5] = np.sqrt(15.0) * y * z
    sh[:, 6] = np.sqrt(5.0) / 2 * (3 * z * z - 1.0)
    sh[:, 7] = np.sqrt(15.0) * x * z
    sh[:, 8] = np.sqrt(15.0) / 2 * (x * x - y * y)
    freqs = np.arange(1, NRAD + 1, dtype=np.float64)
    xr = np.clip(d, 1e-4, 1.0)[:, None].astype(np.float64)
    basis = (np.sqrt(2.0) * np.sin(freqs * np.pi * xr) / xr).astype(np.float32)
    cut = (0.5 * (np.cos(np.pi * np.clip(d, 0.0, 1.0)) + 1.0)).astype(np.float32)
    return (sh * cut[:, None]).astype(np.float32), basis


def partition_graph(receivers):
    import heapq
    deg = np.bincount(receivers, minlength=N_NODES)
    order = np.argsort(-deg, kind="stable")
    nbins = NCORES * NW
    load = np.zeros(nbins, np.int64)
    cnt = np.zeros(nbins, np.int64)
    owner = np.empty(N_NODES, np.int32)
    local = np.empty(N_NODES, np.int32)
    heap = [(0, b) for b in range(nbins)]
    heapq.heapify(heap)
    for n in order:
        while True:
            l, b = heapq.heappop(heap)
            if cnt[b] < 128:
                break
        owner[n] = b // NW
        local[n] = (b % NW) * 128 + cnt[b]
        cnt[b] += 1
        load[b] += deg[n]
        if cnt[b] < 128:
            heapq.heappush(heap, (int(load[b]), b))
    # swap refinement: drive every (core,window) bin's edge load to <= the
    # ceiling average so tiles-per-window hits its floor (any valid
    # partition is correct; this only improves balance).
    target = int(np.ceil(deg.sum() / nbins))
    binid = owner.astype(np.int64) * NW + local // 128
    load = np.zeros(nbins, np.int64)
    np.add.at(load, binid, deg)
    nodes_in = [list(np.where(binid == b)[0]) for b in range(nbins)]
    for _ in range(400):
        bmax = int(np.argmax(load))
        surplus = int(load[bmax] - target)
        if surplus <= 0:
            break
        swapped = False
        for b2 in np.argsort(load):
            b2 = int(b2)
            room = int(target - load[b2])
            if room <= 0:
                break
            cap = min(surplus, room)
            best = None
            for u in nodes_in[bmax]:
                du = int(deg[u])
                for v in nodes_in[b2]:
                    d = du - int(deg[v])
                    if 1 <= d <= cap and (best is None or d > best[0]):
                        best = (d, u, v)
                        if d == cap:
                            break
                if best and best[0] == cap:
                    break
            if best:
                d, u, v = best
                owner[u], owner[v] = owner[v], owner[u]
                local[u], local[v] = local[v], local[u]
                binid[u], binid[v] = binid[v], binid[u]
                nodes_in[bmax].remove(u)
                nodes_in[b2].remove(v)
                nodes_in[bmax].append(v)
                nodes_in[b2].append(u)
                load[bmax] -= d
                load[b2] += d
                swapped = True
                break
        if not swapped:
            break
    nodes_of = np.empty((NCORES, NPC), np.int64)
    for n in range(N_NODES):
        nodes_of[owner[n], local[n]] = n
    return owner, local, nodes_of, int(load.max())


def build_core_edges(receivers, owner, local, tpw):
    T = NW * tpw
    perm = np.full((NCORES, T * 128), -1, np.int64)
    for k in range(NCORES):
        eids = np.where(owner[receivers] == k)[0]
        lr = local[receivers[eids]]
        o = np.argsort(lr, kind="stable")
        eids, lr = eids[o], lr[o]
        w_of = lr // 128
        for w in range(NW):
            sel = eids[w_of == w]
            assert len(sel) <= tpw * 128, "tiles-per-window overflow"
            base = w * tpw * 128
            perm[k, base:base + len(sel)] = sel
    return perm


# ---------------- bass kernel builder (v2, bf16) ----------------
def build_layer_kernel(layer2, T, debug=False):
    import concourse.bass as bass
    import concourse.bacc as bacc
    import concourse.tile as tile
    import concourse.mybir as mybir
    from contextlib import ExitStack

    fp32 = mybir.dt.float32
    bf16 = mybir.dt.bfloat16
    AF = mybir.ActivationFunctionType
    ALU = mybir.AluOpType

    NPATH = 15 if layer2 else 3
    GCOL = GCOL2 if layer2 else GCOL1
    WCOL = NPATH * C
    E_PAD = T * 128
    NG = T // BP
    assert T % BP == 0 and T % NW == 0
    tpw = T // NW
    MAXBLK = max(len(b) for b, _, _ in TP_GROUPS_L2) if layer2 else 0

    nc = bacc.Bacc("TRN2", target_bir_lowering=False)

    ftab = nc.dram_tensor("ftab", [N_NODES, GCOL], bf16, kind="ExternalInput")
    sidx = nc.dram_tensor("sidx", [128, E_PAD // 16], mybir.dt.int16,
                          kind="ExternalInput")
    basT_d = nc.dram_tensor("basisT", [NG, 8, BP * 128], bf16,
                            kind="ExternalInput")
    # 9 selector variants: sh_j-scaled (L2) / cg*sh_j(k)-scaled per k (L1)
    smatv_d = nc.dram_tensor("smatv", [128, T, 9, 128], bf16,
                             kind="ExternalInput")
    oldT_d = nc.dram_tensor("oldT", [64, 9 * NPC], fp32, kind="ExternalInput")
    w1_d = nc.dram_tensor("w1", [8, H], bf16, kind="ExternalInput")
    b1_d = nc.dram_tensor("b1", [H, 1], fp32, kind="ExternalInput")
    w2_d = nc.dram_tensor("w2", [H, H], bf16, kind="ExternalInput")
    b2_d = nc.dram_tensor("b2", [H, 1], fp32, kind="ExternalInput")
    w3_d = nc.dram_tensor("w3", [H, WCOL], bf16, kind="ExternalInput")
    lin_d = [nc.dram_tensor(f"lin{l}", [C, C], bf16, kind="ExternalInput")
             for l in range(3)]
    gw_d = [nc.dram_tensor(f"gw{l}", [C, C], bf16, kind="ExternalInput")
            for l in range(2)]
    gb_d = [nc.dram_tensor(f"gb{l}", [C, 1], fp32, kind="ExternalInput")
            for l in range(2)]
    newT_d = nc.dram_tensor("newT", [64, 9 * NPC], fp32,
                            kind="ExternalOutput")
    if debug:
        dbg_xg = nc.dram_tensor("dbg_xg", [128, BP, GCOL], fp32,
                                kind="ExternalOutput")
        dbg_w = nc.dram_tensor("dbg_w", [128, BP, WCOL], fp32,
                               kind="ExternalOutput")
        dbg_msgs = nc.dram_tensor("dbg_msgs", [128, BP, F], fp32,
                                  kind="ExternalOutput")
        dbg_agg = nc.dram_tensor("dbg_agg", [128, NW, 5 * 128], fp32,
                                 kind="ExternalOutput")
        dbg_h2 = nc.dram_tensor("dbg_h2", [H, BP * 128], fp32,
                                kind="ExternalOutput")

    with tile.TileContext(nc) as tc, ExitStack() as ctx:
        consts = ctx.enter_context(tc.tile_pool(name="consts", bufs=1))
        idx_sb = consts.tile([128, E_PAD // 16], mybir.dt.int16)
        nc.sync.dma_start(idx_sb[:], sidx[:])
        w1_sb = consts.tile([8, H], bf16)
        nc.sync.dma_start(w1_sb[:], w1_d[:])
        b1_sb = consts.tile([H, 1], fp32)
        nc.sync.dma_start(b1_sb[:], b1_d[:])
        w2_sb = consts.tile([H, H], bf16)
        nc.sync.dma_start(w2_sb[:], w2_d[:])
        b2_sb = consts.tile([H, 1], fp32)
        nc.sync.dma_start(b2_sb[:], b2_d[:])
        w3_sb = consts.tile([H, WCOL], bf16)
        nc.sync.dma_start(w3_sb[:], w3_d[:])
        # lin weights replicated into both partition halves (pair-layout agg)
        lin_sb = [consts.tile([128, C], bf16, name=f"lin{l}", tag=f"lin{l}")
                  for l in range(3)]
        for l in range(3):
            nc.sync.dma_start(lin_sb[l][0:C, :], lin_d[l][:])
            nc.sync.dma_start(lin_sb[l][C:2 * C, :], lin_d[l][:])
        gw_sb = [consts.tile([C, C], bf16, name=f"gw{l}", tag=f"gw{l}")
                 for l in range(2)]
        gb_sb = [consts.tile([C, 1], fp32, name=f"gb{l}", tag=f"gb{l}")
                 for l in range(2)]
        for l in range(2):
            nc.sync.dma_start(gw_sb[l][:], gw_d[l][:])
            nc.sync.dma_start(gb_sb[l][:], gb_d[l][:])
        agg_sb = consts.tile([128, NW, 5 * 128], bf16)

        iop = ctx.enter_context(tc.tile_pool(name="iop", bufs=2))
        wp = ctx.enter_context(tc.tile_pool(name="wp", bufs=2))
        pp = ctx.enter_context(tc.tile_pool(name="pp", bufs=1))
        zp = ctx.enter_context(tc.tile_pool(name="zp", bufs=6))
        msgp = ctx.enter_context(tc.tile_pool(name="msgp", bufs=1))

        def issue_gather(g, xg):
            nc.gpsimd.dma_gather(
                out_ap=xg[:],
                in_ap=ftab[:],
                idxs_ap=idx_sb[:, g * (BP * 8):(g + 1) * (BP * 8)],
                num_idxs=BP * 128,
                num_idxs_reg=BP * 128,
                elem_size=GCOL,
            )

        with ExitStack() as psctx:
            mlp_ps = psctx.enter_context(
                tc.tile_pool(name="mlp_ps", bufs=2, space="PSUM"))
            wps_ps = psctx.enter_context(
                tc.tile_pool(name="wps_ps", bufs=2, space="PSUM"))
            agg_pool = psctx.enter_context(
                tc.tile_pool(name="agg_ps", bufs=2, space="PSUM"))

            upt = ctx.enter_context(tc.tile_pool(name="upt", bufs=2))

            def aw_of(w, kg):
                p0 = (kg % 2) * 64
                return agg_sb[p0:p0 + 64, w,
                              (kg // 2) * 128:(kg // 2 + 1) * 128]

            def emit_update(w, ps):
                # node update for window w, inline at window close; the agg
                # PSUM tile's own regions (partitions 0-63) serve as matmul
                # scratch now that agg_sb holds the aggregation.
                def reg(i):
                    return ps[0:C, i * 128:(i + 1) * 128]
                oldw = upt.tile([64, 9, 128], fp32, name="oldw", tag="oldw")
                nc.sync.dma_start(
                    oldw[:], oldT_d[:, :].rearrange(
                        "p (q n) -> p q n",
                        q=9)[:, :, w * 128:(w + 1) * 128])
                neww = upt.tile([64, 9, 128], fp32, name="neww", tag="neww")
                y0p = reg(0)
                nc.tensor.matmul(y0p, lin_sb[0][0:C, :], aw_of(w, 0),
                                 start=True, stop=True)
                y0g = upt.tile([C, 128], fp32, name="y0g", tag="y0g")
                nc.scalar.activation(y0g[:], y0p, AF.Sigmoid)
                y0s = upt.tile([C, 128], fp32, name="y0s", tag="y0s")
                nc.vector.tensor_tensor(out=y0s[:], in0=y0p, in1=y0g[:],
                                        op=ALU.mult)
                x0n = upt.tile([C, 128], fp32, name="x0n", tag="x0n")
                nc.vector.tensor_tensor(out=x0n[:], in0=y0s[:],
                                        in1=oldw[:, 0, :], op=ALU.add)
                x0b = upt.tile([C, 128], bf16, name="x0b", tag="x0b")
                nc.vector.tensor_copy(out=x0b[:], in_=x0n[:])
                nc.vector.tensor_copy(out=neww[:, 0, :], in_=x0n[:])
                gts = []
                for l in (1, 2):
                    gp = reg(l)
                    nc.tensor.matmul(gp, gw_sb[l - 1][:], x0b[:],
                                     start=True, stop=True)
                    gt = upt.tile([C, 128], fp32, name=f"gt{l}", tag=f"gt{l}")
                    nc.scalar.activation(gt[:], gp, AF.Sigmoid,
                                         bias=gb_sb[l - 1][:, 0:1])
                    gts.append(gt)
                for kg in range(1, 9):
                    l = 1 if kg <= 3 else 2
                    p0 = (kg % 2) * C
                    yp = reg(3 + kg % 2)
                    nc.tensor.matmul(yp, lin_sb[l][p0:p0 + C, :],
                                     aw_of(w, kg), start=True, stop=True)
                    gy = upt.tile([C, 128], fp32, name="gy", tag="gy")
                    nc.vector.tensor_tensor(out=gy[:], in0=yp,
                                            in1=gts[l - 1][:], op=ALU.mult)
                    nc.vector.tensor_tensor(out=neww[:, kg, :],
                                            in0=gy[:], in1=oldw[:, kg, :],
                                            op=ALU.add)
                nc.sync.dma_start(
                    newT_d[:, :].rearrange("p (q n) -> p q n",
                                           q=9)[:, :, w * 128:(w + 1) * 128],
                    neww[:])

            agg_open = {}
            xg_tiles = {}
            xg_tiles[0] = iop.tile([128, BP, GCOL], bf16, name="xg", tag="xg")
            issue_gather(0, xg_tiles[0])
            w_tiles = {}

            def emit_w(g):
                # radial MLP + w3 projection for group g (PE, one group ahead)
                bas = iop.tile([8, BP * 128], bf16, name="bas", tag="bas")
                nc.sync.dma_start(bas[:], basT_d[g, :, :])
                h1s = iop.tile([H, BP * 128], bf16, name="h1s", tag="h1s")
                h2s = iop.tile([H, BP * 128], bf16, name="h2s", tag="h2s")
                for c0 in range(0, BP * 128, 512):
                    h1p = mlp_ps.tile([H, 512], fp32, name="h1p", tag="hps")
                    nc.tensor.matmul(h1p[:], w1_sb[:],
                                     bas[:, c0:c0 + 512], start=True, stop=True)
                    sg = iop.tile([H, 512], fp32, name="sg", tag="sg")
                    nc.scalar.activation(sg[:], h1p[:], AF.Sigmoid,
                                         bias=b1_sb[:, 0:1])
                    nc.vector.scalar_tensor_tensor(
                        out=h1s[:, c0:c0 + 512], in0=h1p[:],
                        scalar=b1_sb[:, 0:1], in1=sg[:],
                        op0=ALU.add, op1=ALU.mult)
                    h2p = mlp_ps.tile([H, 512], fp32, name="h2p", tag="hps")
                    nc.tensor.matmul(h2p[:], w2_sb[:],
                                     h1s[:, c0:c0 + 512], start=True, stop=True)
                    sg2 = iop.tile([H, 512], fp32, name="sg2", tag="sg")
                    nc.scalar.activation(sg2[:], h2p[:], AF.Sigmoid,
                                         bias=b2_sb[:, 0:1])
                    nc.vector.scalar_tensor_tensor(
                        out=h2s[:, c0:c0 + 512], in0=h2p[:],
                        scalar=b2_sb[:, 0:1], in1=sg2[:],
                        op0=ALU.add, op1=ALU.mult)
                w_sb = wp.tile([128, BP, WCOL], bf16, name="w_sb", tag="wsb")
                for bt in range(BP):
                    for ci, c0 in enumerate(range(0, WCOL, 480)):
                        c1 = min(c0 + 480, WCOL)
                        w_ps = wps_ps.tile([128, c1 - c0], fp32, name="w_ps",
                                           tag="wps")
                        nc.tensor.matmul(w_ps[:],
                                         h2s[:, bt * 128:(bt + 1) * 128],
                                         w3_sb[:, c0:c1], start=True, stop=True)
                        nc.scalar.copy(w_sb[:, bt, c0:c1], w_ps[:])
                w_tiles[g] = w_sb

            emit_w(0)

            for g in range(NG):
                t0 = g * BP
                xg = xg_tiles.pop(g)
                if g + 1 < NG:
                    xg_tiles[g + 1] = iop.tile([128, BP, GCOL], bf16, name="xg", tag="xg")
                    issue_gather(g + 1, xg_tiles[g + 1])
                    emit_w(g + 1)
                w_sb = w_tiles.pop(g)
                smtv = iop.tile([128, BP, 9, 128], bf16, tag="smtv")
                nc.sync.dma_start(smtv[:], smatv_d[:, t0:t0 + BP, :, :])


                if layer2:
                    # tensor product, PE-fold form: all CG triples become
                    # matmuls of signed-|cg|-scaled P-blocks against the
                    # sh_j-scaled selector variants, accumulated in PSUM.
                    P_tiles = {}
                    for gi, (blocks, jlist, triples) in enumerate(TP_GROUPS_L2):
                        P_sb = pp.tile([128, BP, len(blocks) * C], bf16,
                                       name="psb", tag=f"psb{gi}")
                        for nb, (p, ig) in enumerate(blocks):
                            eng = (nc.gpsimd if (gi == 2 and ig >= 4)
                                   or (gi == 1 and ig >= 7)
                                   else nc.vector)
                            eng.tensor_tensor(
                                out=P_sb[:, :, nb * C:(nb + 1) * C],
                                in0=xg[:, :, ig * C:(ig + 1) * C],
                                in1=w_sb[:, :, p * C:(p + 1) * C],
                                op=ALU.mult)
                        P_tiles[gi] = P_sb

                    def agg_of(w):
                        if w not in agg_open:
                            agg_open[w] = (
                                agg_pool.tile([128, 5 * 128], fp32,
                                              name="aggps", tag="aggps"),
                                {})
                        return agg_open[w]

                    for j in range(1, 9):
                        gi, slots = FOLD_SLOTS[j]
                        P_sb = P_tiles[gi]
                        for (m, ev, od) in slots:
                            pt = zp.tile([128, BP, 128], bf16,
                                         name="pt", tag="pt")
                            if ev is not None:
                                z, v = ev
                                nc.vector.tensor_scalar_mul(
                                    pt[:, :, 0:C],
                                    P_sb[:, :, z * C:(z + 1) * C], v)
                            if od is not None:
                                z, v = od
                                nc.vector.tensor_scalar_mul(
                                    pt[:, :, C:2 * C],
                                    P_sb[:, :, z * C:(z + 1) * C], v)
                            for bt in range(BP):
                                t = t0 + bt
                                w = t // tpw
                                ps, started = agg_of(w)
                                bank = "B" if m == 4 else "A"
                                if ev is not None and od is not None:
                                    halves = (0, 1)
                                    stat = pt[:, bt, :]
                                    out = ps[:, m * 128:(m + 1) * 128]
                                elif ev is not None:
                                    halves = (0,)
                                    stat = pt[:, bt, 0:C]
                                    out = ps[0:C, m * 128:(m + 1) * 128]
                                else:
                                    halves = (1,)
                                    stat = pt[:, bt, C:2 * C]
                                    out = ps[C:2 * C,
                                             m * 128:(m + 1) * 128]
                                st = not any(started.get((bank, h))
                                             for h in halves)
                                for h in halves:
                                    started[(bank, h)] = True
                                nc.tensor.matmul(
                                    out, stat, smtv[:, bt, j, :],
                                    start=st, stop=False,
                                    skip_group_check=True)
                    # l2=0 group direct (cg folded into w3), closes groups
                    for bt in range(BP):
                        t = t0 + bt
                        w, t_in_w = t // tpw, t % tpw
                        ps, started = agg_of(w)
                        last = t_in_w == tpw - 1
                        for pr in range(5):
                            ncols = 128 if pr < 4 else 64
                            bank = "B" if pr == 4 else "A"
                            halves = (0, 1) if ncols == 128 else (0,)
                            st = not any(started.get((bank, h))
                                         for h in halves)
                            for h in halves:
                                started[(bank, h)] = True
                            nc.tensor.matmul(
                                ps[0:ncols, pr * 128:(pr + 1) * 128],
                                P_tiles[0][:, bt,
                                           pr * 128:pr * 128 + ncols],
                                smtv[:, bt, 0, :], start=st,
                                stop=last and pr >= 3,
                                skip_group_check=True)
                    for bt in range(BP):
                        t = t0 + bt
                        w, t_in_w = t // tpw, t % tpw
                        if t_in_w == tpw - 1:
                            ps, _ = agg_open.pop(w)
                            nc.vector.tensor_copy(out=agg_sb[:, w, :],
                                                  in_=ps[:])
                            emit_update(w, ps)
                else:
                    # layer 1: per-k path products, then one fold matmul per
                    # k against the cg-baked selector variant (no msgs)
                    P_sb = pp.tile([128, BP, 9 * C], bf16, tag="psb")
                    for kg in range(9):
                        p = L1_PATH_OF_K[kg]
                        nc.vector.tensor_tensor(
                            out=P_sb[:, :, kg * C:(kg + 1) * C],
                            in0=xg[:, :, 0:C],
                            in1=w_sb[:, :, p * C:(p + 1) * C],
                            op=ALU.mult)
                    for bt in range(BP):
                        t = t0 + bt
                        w, t_in_w = t // tpw, t % tpw
                        if w not in agg_open:
                            agg_open[w] = (agg_pool.tile(
                                [128, 5 * 128], fp32, name="aggps",
                                tag="aggps"), {})
                        ps, started = agg_open[w]
                        last = t_in_w == tpw - 1
                        for kg in range(9):
                            m = kg // 2
                            p0 = (kg % 2) * C
                            bank = "B" if m == 4 else "A"
                            st = not started.get((bank, kg % 2))
                            started[(bank, kg % 2)] = True
                            nc.tensor.matmul(
                                ps[p0:p0 + C, m * 128:(m + 1) * 128],
                                P_sb[:, bt, kg * C:(kg + 1) * C],
                                smtv[:, bt, kg, :], start=st,
                                stop=last and kg >= 7,
                                skip_group_check=True)
                        if last:
                            ps, _ = agg_open.pop(w)
                            nc.vector.tensor_copy(out=agg_sb[:, w, :],
                                                  in_=ps[:])
                            emit_update(w, ps)

                if debug and g == 0:
                    dxg = msgp.tile([128, BP, GCOL], fp32, tag="dxg")
                    nc.vector.tensor_copy(out=dxg[:], in_=xg[:])
                    nc.sync.dma_start(dbg_xg[:], dxg[:])
                    dw = msgp.tile([128, BP, WCOL], fp32, tag="dw")
                    nc.vector.tensor_copy(out=dw[:], in_=w_sb[:])
                    nc.sync.dma_start(dbg_w[:], dw[:])
                    dh = msgp.tile([H, BP * 128], fp32, tag="dh")
                    nc.vector.tensor_copy(out=dh[:], in_=h2s[:])
                    nc.sync.dma_start(dbg_h2[:], dh[:])

        if debug:
            dbg_agg_f = None

    nc.compile()
    return nc


# ---------------- host orchestration ----------------
def _chunked_T(feats_own):
    """[NPC, 576] -> kg-blocked transposed [64, 9*NPC]."""
    out = np.empty((64, 9 * NPC), np.float32)
    for kg in range(9):
        out[:, kg * NPC:(kg + 1) * NPC] = feats_own[:, kg * 64:(kg + 1) * 64].T
    return out


def _unchunk_T(newT):
    """[64, 9*NPC] -> [NPC, 576]."""
    out = np.empty((NPC, 576), np.float32)
    for kg in range(9):
        out[:, kg * 64:(kg + 1) * 64] = newT[:, kg * NPC:(kg + 1) * NPC].T
    return out


_CACHE = {}


def _prep(positions, senders, receivers):
    key = (senders.tobytes(), receivers.tobytes(), positions.tobytes())
    if _CACHE.get("key") == key:
        return _CACHE["val"]
    sh_eff, basis = edge_geometry(positions, senders, receivers)
    owner, local, nodes_of, _ = partition_graph(receivers)
    # tiles per window: max bin edge count, rounded to tiles, even for BP
    deg_bin = np.zeros(NCORES * NW, np.int64)
    np.add.at(deg_bin, owner[receivers] * NW + local[receivers] // 128, 1)
    tpw = (int(deg_bin.max()) + 127) // 128
    while (NW * tpw) % BP:
        tpw += 1
    T = NW * tpw
    perm = build_core_edges(receivers, owner, local, tpw)

    valid = perm >= 0
    eg = np.where(valid, perm, 0)
    snd = np.where(valid, senders[eg], 0).astype(np.int16)      # [NC, T*128]
    shp_e = sh_eff[eg] * valid[..., None]                        # [NC, T*128, 9]
    bas_e = basis[eg] * valid[..., None]                         # [NC, T*128, 8]
    lr = np.where(valid, local[receivers[eg]], 0)

    NG = T // BP
    inv = np.float32(1.0 / np.sqrt(AVG_NN))
    sidx = np.empty((NCORES, 128, T * 128 // 16), np.int16)
    shp_h = np.empty((NCORES, 128, T, 9), np.float32)
    shp1_h = None  # filled after shp_h below
    bas_h = np.empty((NCORES, NG, 8, BP * 128), np_bf16)
    smat_h = np.zeros((NCORES, 128, T, 128), np_bf16)
    smatv_h = np.zeros((NCORES, 128, T, 9, 128), np_bf16)
    smatv1_h = np.zeros((NCORES, 128, T, 9, 128), np_bf16)
    cg1 = np.asarray(L1_CG_OF_K, np.float32)
    for k in range(NCORES):
        s = snd[k].reshape(T * 8, 16)
        sidx[k] = np.tile(s.T, (8, 1))
        shp_h[k] = shp_e[k].reshape(T, 128, 9).transpose(1, 0, 2)
        if shp1_h is None:
            shp1_h = np.empty((NCORES, 128, T, 9), np.float32)
        shp1_h[k] = shp_h[k] * cg1
        bas_h[k] = bas_e[k].reshape(NG, BP * 128, 8).transpose(0, 2, 1).astype(np_bf16)
        v = valid[k]
        e_slots = np.arange(T * 128)
        p_, t_ = e_slots % 128, e_slots // 128
        cols = lr[k] - (t_ // tpw) * 128
        ok = v & (cols >= 0) & (cols < 128)
        sm = np.zeros((128, T, 128), np.float32)
        sm[p_[ok], t_[ok], cols[ok]] = inv
        smat_h[k] = sm.astype(np_bf16)
        smatv_h[k] = (sm[:, :, None, :]
                      * shp_h[k][:, :, :, None]).astype(np_bf16)
        smatv1_h[k] = (sm[:, :, None, :]
                       * shp1_h[k][:, :, :, None]).astype(np_bf16)
    val = dict(T=T, NG=NG, tpw=tpw, nodes_of=nodes_of, sidx=sidx,
               shp_h=shp_h, shp1_h=shp1_h, bas_h=bas_h, smat_h=smat_h,
               smatv_h=smatv_h, smatv1_h=smatv1_h)
    _CACHE["key"], _CACHE["val"] = key, val
    return val


PROFILE = False          # set True by test.py to capture timing
PROF_NS = []             # per-launch exec_time_ns when PROFILE
TRACE_DIRS = []          # per-launch trace dirs when PROFILE


def _run_layer(nc, pre, table_bf16, oldT_by_core, lw, layer2):
    from concourse.bass_utils import run_bass_kernel_spmd
    in_maps = []
    for k in range(NCORES):
        m = dict(ftab=table_bf16,
                 sidx=pre["sidx"][k],
                 basisT=pre["bas_h"][k],
                 oldT=oldT_by_core[k],
                 w1=lw["w1"], b1=lw["b1"], w2=lw["w2"], b2=lw["b2"],
                 w3=lw["w3"], lin0=lw["lin"][0], lin1=lw["lin"][1],
                 lin2=lw["lin"][2], gw0=lw["gw"][0], gw1=lw["gw"][1],
                 gb0=lw["gb"][0], gb1=lw["gb"][1])
        m["smatv"] = (pre["smatv_h"] if layer2 else pre["smatv1_h"])[k]
        in_maps.append(m)
    if PROFILE:
        import time
        t0 = time.time()
        res = run_bass_kernel_spmd(nc, in_maps, list(range(NCORES)))
        PROF_NS.append(int((time.time() - t0) * 1e9))
    else:
        res = run_bass_kernel_spmd(nc, in_maps, list(range(NCORES)))
    return [res.results[k]["newT"] for k in range(NCORES)]


def _layer_weights(inputs, i, npaths):
    f32 = np.float32
    w3 = np.array(inputs["mlp_w3"][i][:, :npaths * C], f32)
    if npaths == 15:
        # l2=0 paths: CG folded into w3 columns (device skips their zj/triples)
        for p, v in L20_CG.items():
            w3[:, p * C:(p + 1) * C] *= np.float32(v)
    return dict(
        w1=np.ascontiguousarray(inputs["mlp_w1"][i]).astype(np_bf16),
        b1=np.ascontiguousarray(inputs["mlp_b1"][i], f32).reshape(H, 1),
        w2=np.ascontiguousarray(inputs["mlp_w2"][i]).astype(np_bf16),
        b2=np.ascontiguousarray(inputs["mlp_b2"][i], f32).reshape(H, 1),
        w3=np.ascontiguousarray(w3).astype(np_bf16),
        lin=[np.ascontiguousarray(inputs["lin_self"][i, l]).astype(np_bf16)
             for l in range(3)],
        gw=[np.ascontiguousarray(inputs["gate_w"][i, l]).astype(np_bf16)
            for l in range(2)],
        gb=[np.ascontiguousarray(inputs["gate_b"][i, l], f32).reshape(C, 1)
            for l in range(2)],
    )


_KERNEL_CACHE = {}


def _get_kernels(T):
    if T not in _KERNEL_CACHE:
        _KERNEL_CACHE[T] = (build_layer_kernel(False, T),
                            build_layer_kernel(True, T))
    return _KERNEL_CACHE[T]


def kernel(**inputs):
    positions = np.asarray(inputs["positions"], np.float32)
    species = np.asarray(inputs["species"]).astype(np.int64)
    senders = np.asarray(inputs["senders"]).astype(np.int64)
    receivers = np.asarray(inputs["receivers"]).astype(np.int64)

    pre = _prep(positions, senders, receivers)
    T = pre["T"]
    nc1, nc2 = _get_kernels(T)
    nodes_of = pre["nodes_of"]

    # initial features: x0 from species embedding (host; tiny)
    x0 = (np.asarray(inputs["embed"], np.float32)[species]
          @ np.asarray(inputs["w_proj"], np.float32))          # [N, 64]
    table1 = np.zeros((N_NODES, GCOL1), np_bf16)
    table1[:, 0:C] = x0.astype(np_bf16)

    # ---- layer 1 ----
    tbl = np.zeros((N_NODES, F), np.float32)
    tbl[:, 0:C] = x0
    oldT = [_chunked_T(tbl[nodes_of[k]]) for k in range(NCORES)]
    lw = _layer_weights(inputs, 0, 3)
    newT = _run_layer(nc1, pre, table1, oldT, lw, False)

    table2f = np.empty((N_NODES, F), np.float32)
    for k in range(NCORES):
        table2f[nodes_of[k]] = _unchunk_T(newT[k])
    table2 = np.zeros((N_NODES, GCOL2), np_bf16)
    table2[:, 0:F] = table2f.astype(np_bf16)

    # ---- layer 2 ----
    lw = _layer_weights(inputs, 1, 15)
    newT2 = _run_layer(nc2, pre, table2, newT, lw, True)

    table3 = np.empty((N_NODES, F), np.float32)
    for k in range(NCORES):
        table3[nodes_of[k]] = _unchunk_T(newT2[k])

    # ---- output: reorder component-major -> reference layout + alpha ----
    t3 = table3.reshape(N_NODES, 9, C)
    out = np.empty((N_NODES, F), np.float32)
    out[:, 0:64] = t3[:, 0]
    out[:, 64:256] = (0.5 * t3[:, 1:4]).transpose(0, 2, 1).reshape(N_NODES, 192)
    out[:, 256:576] = (0.25 * t3[:, 4:9]).transpose(0, 2, 1).reshape(N_NODES, 320)
    return out
